# revision 36
# baseline (speedup 1.0000x reference)
"""LiteMLA (EfficientViT multi-scale linear attention) Trainium2 Bass kernel.

Sharding: data-parallel over batch B=8 across 8 NeuronCores (1 image/core).
Per-core pipeline:
  1. Streamed per-nt x: pass1 (bf16, natural channel order) -> zero-padded
     SBUF image for conv taps; pass2 -> attention Q buffer + id-scale K/V
     stages. The q channels use Dekker-split bf16 matmuls (whi.xhi +
     whi.xlo + wlo.xhi, ~16-bit effective mantissa): the id-scale heads are
     ill-conditioned (att = (vk@relu q)/(den@relu q) is 0/0 at positions
     where all 8 q dims are negative), so relu(q)'s sign pattern must track
     the fp32 reference closely; plain bf16 or HW-float32r inputs flip
     signs and cost ~0.15 rel err. k/v stay bf16 (4096-term averages).
     s3 conv is emitted interleaved into this loop to fill PE idle.
  2. s3/s5: depthwise 3x3/5x5 + grouped 1x1 FUSED on host into per-tap
     block-diagonal [96,96] weights; all taps of a block fetched in ONE
     sync-queue DMA; PE matmuls accumulate taps in PSUM reading shifted
     slices of the padded image.
  3. relu-linear attention: per spatial tile, relu(k)/v transposed by the
     DMA engines (xbar dma_start_transpose, contiguous [128,128] dst tiles)
     and reduced into per-16-head vk outer products; per-tile PSUM partials
     fold into SBUF accumulators on DVE; denominator = row-sums of relu(k).
  4. apply: att_raw = vk @ relu(q) via block-diagonal apply weights on raw
     Q (no per-position pre-scale, keeping PE free of the normalize chain);
     the batched [96,TN] denominator reciprocal is broadcast-expanded per
     region on PE and applied at the PSUM drain (one DVE multiply); proj
     contracts 128 rows (2 groups per matmul). Each tile's attention stage
     is processed one iteration deferred so its DMA-transpose issues overlap
     the next tile's matmuls.

All SBUF/PSUM operand slices start at partition 0/32/64/96 (HW requirement).
"""

import sys
import numpy as np

sys.path.insert(0, "/opt/trn_rl_repo")

B, CIN, HH, WW = 8, 256, 64, 64
N = HH * WW            # 4096
HEADS = 32             # per scale
C3 = 768
NHEADS = 96
PADW = WW + 4          # 68
NT = 8                 # spatial tiles of 512 positions (8 image rows each)
TN = 512
HALF = 2               # nts processed per conv weight fetch
TAPS3 = [(dy, dx) for dy in (-1, 0, 1) for dx in (-1, 0, 1)]
TAPS5 = [(dy, dx) for dy in (-2, -1, 0, 1, 2) for dx in (-2, -1, 0, 1, 2)]
NBLK = 8               # conv channel blocks of 4 head-groups
BLK = 96
NREG = 6               # vk regions of 16 heads

_cache = {}

PS = 16.0              # fp8 pad image pre-scale
WS = 256.0             # fp8 fused conv weight pre-scale
SC = 1.0 / (PS * WS)   # conv psum drain scale (2^-12)
NU = {3: 5, 5: 13}     # DoubleRow tap-pair units per scale


def _head_of(g12, i):
    return 16 * (g12 // 2) + 8 * (g12 % 2) + i


def _host_weights(inp):
    f32 = np.float32
    W = np.asarray(inp["qkv_w"], f32)[:, :, 0, 0]            # [768, 256]
    qkv_b = np.asarray(inp["qkv_b"], f32)
    pw = {3: np.asarray(inp["pw3_w"], f32)[:, :, 0, 0],
          5: np.asarray(inp["pw5_w"], f32)[:, :, 0, 0]}
    pwb = {3: np.asarray(inp["pw3_b"], f32), 5: np.asarray(inp["pw5_b"], f32)}
    dw = {3: np.asarray(inp["dw3_w"], f32)[:, 0],
          5: np.asarray(inp["dw5_w"], f32)[:, 0]}
    dwb = {3: np.asarray(inp["dw3_b"], f32), 5: np.asarray(inp["dw5_b"], f32)}
    proj_w = np.asarray(inp["proj_w"], f32)[:, :, 0, 0]      # [256, 768]
    proj_b = np.asarray(inp["proj_b"], f32)

    d = {}
    # pass1 weights pre-scaled by PS: pad fp8 image = PS*qkv via a pure-copy
    # drain (no scale op needed)
    d["w1t"] = np.ascontiguousarray(W.T) * PS                # [256, 768]
    # per-head reciprocal scale: attention is scale-invariant in q/k per head
    # and scales linearly with v, so conv drains skip the 1/(PS*WS) descale
    # and the v-scale is folded into the rcb copy (heads 32..95 are conv)
    sv = np.ones((NHEADS, 1), f32)
    sv[32:] = 1.0 / (PS * WS)
    d["svec"] = sv
    # pass1 fp8 DoubleRow weights: [NBLK, 128, 2, 96], *32 (drain scale .5*PS/16)
    w18 = np.empty((NBLK, 128, 2, BLK), f32)
    for b in range(NBLK):
        for j in range(2):
            w18[b, :, j, :] = 32.0 * W[BLK * b:BLK * (b + 1), 128 * j:128 * (j + 1)].T
    d["w18"] = w18
    perm2 = np.empty(768, np.int64)
    for h in range(HEADS):
        for e in range(8):
            perm2[h * 8 + e] = h * 24 + e
            perm2[256 + h * 8 + e] = h * 24 + 8 + e
            perm2[512 + h * 8 + e] = h * 24 + 16 + e
    d["w2t"] = np.ascontiguousarray(W[perm2].T)
    d["bi2"] = qkv_b[perm2].reshape(768, 1)

    # fused conv weights: per tap, 8 blocks of 4 groups, [96in, 96out q|k|v];
    # stored per-block contiguous over taps: [NBLK, 96in, taps*96out]
    for s, taps in ((3, TAPS3), (5, TAPS5)):
        fw = np.zeros((len(taps), NBLK, BLK, BLK), f32)
        fb = np.zeros((NBLK, BLK), f32)
        for b in range(NBLK):
            for gl in range(4):
                g = 4 * b + gl
                M24 = pw[s][g * 24:(g + 1) * 24]             # [24 out(oo), 24 in]
                bias24 = pwb[s][g * 24:(g + 1) * 24] + M24 @ dwb[s][g * 24:(g + 1) * 24]
                for oo in range(24):
                    m = (oo // 8) * 32 + gl * 8 + (oo % 8)   # [q32|k32|v32]
                    fb[b, m] = bias24[oo]
                dvec = dw[s][g * 24:(g + 1) * 24]            # [24 in, kh, kw]
                for ti, (dy, dx) in enumerate(taps):
                    wt = M24 * dvec[:, dy + s // 2, dx + s // 2][None, :]
                    for oo in range(24):
                        m = (oo // 8) * 32 + gl * 8 + (oo % 8)
                        fw[ti, b, gl * 24:(gl + 1) * 24, m] = wt[oo]
        # fp8 DoubleRow pair-stacked: [NBLK, BLK_in, U, 2, BLK_out]; unit u
        # holds taps (2u, 2u+1); odd tap count -> last unit slot 1 zeroed
        U = NU[s]
        fw8 = np.zeros((NBLK, BLK, U, 2, BLK), f32)
        for u in range(U):
            for j in range(2):
                ti = 2 * u + j
                if ti < len(taps):
                    fw8[:, :, u, j, :] = fw[ti] * WS
        d[f"fw{s}"] = fw8
        # conv drain bias in raw-psum units (outputs stay scaled by PS*WS)
        d[f"bc{s}"] = fb.reshape(NBLK, BLK, 1) * (PS * WS)

    d["idt"] = np.eye(128, dtype=f32)

    # masks for vk -> apply-weight assembly (dd-major cols, no den col)
    for half in range(2):
        mp = np.zeros((128, 64), f32)   # [(hp,e), (dd,h)]
        for p in range(128):
            hp = p // 8
            for h in range(8):
                if hp == h + 8 * half:
                    for dd in range(8):
                        mp[p, 8 * dd + h] = 1.0
        d[f"mp{half}"] = mp

    # expand matrices: er maps rc rows (96 heads) onto ap2's row layout
    # (col p: half=p//64, dd, i -> head 16r+8*(p//64)+p%8); mdex places den
    # values for the Q-region row layout (row p -> head 16r + p//8)
    er = np.zeros((NREG, 96, 128), f32)
    mdex = np.zeros((NREG, 128, 96), f32)
    for r in range(NREG):
        for p in range(128):
            er[r, 16 * r + 8 * (p // 64) + p % 8, p] = 1.0
            mdex[r, p, 16 * r + p // 8] = 1.0
    d["er"] = er
    d["mdex"] = mdex

    # proj lhsT [6, 128, 256]: rows 0:64 group 2r, 64:128 group 2r+1; row
    # (64*half + 8*dd + i) = proj col of head _head_of(2r+half, i), dim dd
    PW2 = np.zeros((NREG, 128, 256), f32)
    for r in range(NREG):
        for half in range(2):
            for i in range(8):
                Hh = _head_of(2 * r + half, i)
                for dd in range(8):
                    PW2[r, 64 * half + 8 * dd + i] = proj_w[:, 8 * Hh + dd]
    d["pw2"] = PW2
    d["pb"] = proj_b.reshape(256, 1)
    return d


def _build():
    import concourse.bass as bass
    import concourse.bacc as bacc_mod
    import concourse.mybir as mybir
    from concourse.tile import TileContext

    dt = mybir.dt
    f32, bf16, f8 = dt.float32, dt.bfloat16, dt.float8e4
    AF = mybir.ActivationFunctionType
    ALU = mybir.AluOpType
    AX = mybir.AxisListType
    PM = mybir.MatmulPerfMode

    nc = bacc_mod.Bacc()
    x_in = nc.dram_tensor("xf", [CIN, N], f32, kind="ExternalInput")
    dW2QH = nc.dram_tensor("w2qh", [CIN, 256], bf16, kind="ExternalInput")
    dW2QL = nc.dram_tensor("w2ql", [CIN, 256], bf16, kind="ExternalInput")
    dBI2 = nc.dram_tensor("bi2", [C3, 1], f32, kind="ExternalInput")
    dFW3 = nc.dram_tensor("fw3", [NBLK, BLK, NU[3], 2, BLK], f8, kind="ExternalInput")
    dFW5 = nc.dram_tensor("fw5", [NBLK, BLK, NU[5], 2, BLK], f8, kind="ExternalInput")
    dBC3 = nc.dram_tensor("bc3", [NBLK, BLK, 1], f32, kind="ExternalInput")
    dBC5 = nc.dram_tensor("bc5", [NBLK, BLK, 1], f32, kind="ExternalInput")
    dIDT = nc.dram_tensor("idt", [128, 128], bf16, kind="ExternalInput")
    dMP = [nc.dram_tensor(f"mp{h}", [128, 64], bf16, kind="ExternalInput") for h in range(2)]
    dER = nc.dram_tensor("er", [NREG, 96, 128], bf16, kind="ExternalInput")
    dMDEX = nc.dram_tensor("mdex", [NREG, 128, 96], bf16, kind="ExternalInput")
    dPW2 = nc.dram_tensor("pw2", [NREG, 128, 256], bf16, kind="ExternalInput")
    dPB = nc.dram_tensor("pb", [256, 1], f32, kind="ExternalInput")
    dSV = nc.dram_tensor("svec", [NHEADS, 1], f32, kind="ExternalInput")
    dW18 = nc.dram_tensor("w18", [NBLK, 128, 2, BLK], f8, kind="ExternalInput")
    dW2kv = nc.dram_tensor("w2kv", [CIN, 512], bf16, kind="ExternalInput")
    d_out = nc.dram_tensor("out", [CIN, N], f32, kind="ExternalOutput")

    with TileContext(nc) as tc:
        with (
            tc.tile_pool(name="consts", bufs=1) as cpool,
            tc.tile_pool(name="persist", bufs=1) as qpool,
            tc.tile_pool(name="wstream", bufs=2) as wpool,
            tc.tile_pool(name="stage", bufs=2) as spool,
        ):
            # ---- constants (off the sync queue so x DMAs start immediately) --
            w2qh = [cpool.tile([128, 256], bf16, name=f"w2qh_{k}") for k in range(2)]
            w2ql = [cpool.tile([128, 256], bf16, name=f"w2ql_{k}") for k in range(2)]
            for k in range(2):
                nc.scalar.dma_start(out=w2qh[k][:], in_=dW2QH[128 * k:128 * (k + 1), :])
                nc.scalar.dma_start(out=w2ql[k][:], in_=dW2QL[128 * k:128 * (k + 1), :])
            bi2 = [cpool.tile([128, 1], f32, name=f"bi2_{j}") for j in range(6)]
            for j in range(6):
                nc.scalar.dma_start(out=bi2[j][:], in_=dBI2[128 * j:128 * (j + 1), :])
            bc = {}
            for s, db in ((3, dBC3), (5, dBC5)):
                bc[s] = [cpool.tile([BLK, 1], f32, name=f"bc{s}_{b}") for b in range(NBLK)]
                for b in range(NBLK):
                    nc.gpsimd.dma_start(out=bc[s][b][:], in_=db[b])
            # resident fp8 DoubleRow conv weights: [96, U, 2, 96] per block
            cw = {}
            for s, dfw in ((3, dFW3), (5, dFW5)):
                cw[s] = [cpool.tile([BLK, NU[s], 2, BLK], f8, name=f"cw{s}_{b}")
                         for b in range(NBLK)]
                for b in range(NBLK):
                    eng = nc.scalar if b % 2 == 0 else nc.gpsimd
                    eng.dma_start(out=cw[s][b][:], in_=dfw[b])
            idt = cpool.tile([128, 128], bf16, name="idt")
            nc.scalar.dma_start(out=idt[:], in_=dIDT[:, :])
            mp = [cpool.tile([128, 64], bf16, name=f"mp_{h}") for h in range(2)]
            for h in range(2):
                nc.gpsimd.dma_start(out=mp[h][:], in_=dMP[h][:, :])
            ert = [cpool.tile([96, 128], bf16, name=f"er_{r}") for r in range(NREG)]
            mdex = [cpool.tile([128, 96], bf16, name=f"mdex_{r}") for r in range(NREG)]
            pwt = [cpool.tile([128, 256], bf16, name=f"pw2_{r}") for r in range(NREG)]
            for r in range(NREG):
                nc.gpsimd.dma_start(out=ert[r][:], in_=dER[r])
                nc.gpsimd.dma_start(out=mdex[r][:], in_=dMDEX[r])
                nc.gpsimd.dma_start(out=pwt[r][:], in_=dPW2[r])
            pbt = [cpool.tile([128, 1], f32, name=f"pbt_{m}") for m in range(2)]
            for m in range(2):
                nc.gpsimd.dma_start(out=pbt[m][:], in_=dPB[128 * m:128 * (m + 1), :])
            svt = cpool.tile([NHEADS, 1], f32, name="svt")
            nc.gpsimd.dma_start(out=svt[:], in_=dSV[:, :])
            w18t = cpool.tile([128, NBLK, 2, BLK], f8, name="w18t")
            for b in range(NBLK):
                nc.scalar.dma_start(out=w18t[:, b], in_=dW18[b])
            w2kv = [cpool.tile([128, 512], bf16, name=f"w2kv_{k}") for k in range(2)]
            for k in range(2):
                nc.scalar.dma_start(out=w2kv[k][:], in_=dW2kv[128 * k:128 * (k + 1), :])
            epsw = cpool.tile([1, 96], bf16, name="epsw")
            nc.gpsimd.memset(epsw[:], 1e-15)
            ones1 = cpool.tile([1, TN], bf16, name="ones1")
            nc.gpsimd.memset(ones1[:], 1.0)

            # ---- persistent activations ----
            pad = [qpool.tile([BLK, PADW, PADW], f8, name=f"pad_{b}") for b in range(NBLK)]
            for b in range(NBLK):
                # zero only the 2-wide borders; interior is fully written by
                # pass 1 (keeps these memsets off pass 1's dependency chain)
                nc.gpsimd.memset(pad[b][:, 0:2, :], 0.0)
                nc.gpsimd.memset(pad[b][:, PADW - 2:PADW, :], 0.0)
                nc.gpsimd.memset(pad[b][:, 2:PADW - 2, 0:2], 0.0)
                nc.gpsimd.memset(pad[b][:, 2:PADW - 2, PADW - 2:PADW], 0.0)
            Q = [qpool.tile([128, N], bf16, name=f"Q_{r}") for r in range(NREG)]
            kpart = [qpool.tile([128, NT], f32, name=f"kpart_{r}") for r in range(NREG)]
            vks_sb = [qpool.tile([128, 128], bf16, name=f"vks_{r}") for r in range(NREG)]

            vks_acc = [qpool.tile([128, 128], f32, name=f"vka_{r}")
                       for r in range(NREG)]

            # PSUM pool A: phases 1 + conv (reclaimed before apply phase)
            _pA = tc.tile_pool(name="psumA", bufs=2, space="PSUM")
            ppool = _pA.__enter__()

            # ============ shared per-tile attention stage ====================
            def process_stage(s_idx, nt, ks, vs):
                # first two id-stages: PE transposes (PE is idle pre-conv and
                # this keeps DMA-transfer latency off the early critical path)
                pe_tr = (s_idx == 0 and nt <= 1)
                """ks/vs: 2 bf16 [128,512] stage tiles (relu'd k / raw v)."""
                for t in range(2):
                    r = 2 * s_idx + t
                    nc.vector.reduce_sum(out=kpart[r][:, nt:nt + 1], in_=ks[t][:], axis=AX.X)
                vkps = [ppool.tile([128, 128], f32, name="vkps", tag="vk", bufs=2)
                        for _ in range(2)]
                for jj in range(4):
                    for t in range(2):
                        kT = spool.tile([128, 128], bf16, name="kT", tag="kT", bufs=12)
                        vT = spool.tile([128, 128], bf16, name="vT", tag="vT", bufs=12)
                        if pe_tr:
                            tp = ppool.tile([128, 128], bf16, name="tp", tag="mm",
                                            bufs=3)
                            nc.tensor.transpose(
                                tp[:], ks[t][:, 128 * jj:128 * (jj + 1)], idt[:])
                            nc.scalar.copy(out=kT[:], in_=tp[:])
                            tp2 = ppool.tile([128, 128], bf16, name="tp2", tag="mm",
                                             bufs=3)
                            nc.tensor.transpose(
                                tp2[:], vs[t][:, 128 * jj:128 * (jj + 1)], idt[:])
                            nc.vector.tensor_copy(out=vT[:], in_=tp2[:])
                        else:
                            nc.sync.dma_start_transpose(
                                out=kT[:], in_=ks[t][:, 128 * jj:128 * (jj + 1)])
                            nc.sync.dma_start_transpose(
                                out=vT[:], in_=vs[t][:, 128 * jj:128 * (jj + 1)])
                        nc.tensor.matmul(vkps[t][:], kT[:], vT[:],
                                         start=(jj == 0), stop=(jj == 3))
                for t in range(2):
                    r = 2 * s_idx + t
                    if nt == 0:
                        nc.vector.tensor_copy(out=vks_acc[r][:], in_=vkps[t][:])
                    else:
                        nc.vector.tensor_tensor(out=vks_acc[r][:], in0=vks_acc[r][:],
                                                in1=vkps[t][:], op=ALU.add)
                    if nt == NT - 1:
                        nc.gpsimd.tensor_copy(out=vks_sb[r][:], in_=vks_acc[r][:])

            # ===== pass 1+2 per nt: stream x, build pad image + id stage =====

            def emit_pass12(nt):
                xt = [spool.tile([128, TN], f32, name="xt", tag=f"xt{k}", bufs=2)
                      for k in range(2)]
                xb = [spool.tile([128, TN], bf16, name="xb", tag=f"xb{k}", bufs=2)
                      for k in range(2)]
                xl = [spool.tile([128, TN], bf16, name="xl", tag=f"xl{k}", bufs=2)
                      for k in range(2)]
                xf8 = spool.tile([128, 2, TN], f8, name="xf8", tag="xf8", bufs=2)
                for k in range(2):
                    nc.sync.dma_start(out=xt[k][:],
                                      in_=x_in[128 * k:128 * (k + 1), TN * nt:TN * (nt + 1)])
                    nc.gpsimd.tensor_copy(out=xb[k][:], in_=xt[k][:])
                    nc.gpsimd.tensor_tensor(out=xl[k][:], in0=xt[k][:], in1=xb[k][:],
                                            op=ALU.subtract)
                    nc.gpsimd.tensor_copy(out=xf8[:, k, :], in_=xt[k][:])
                # pass 1 (fp8 DoubleRow): natural order -> padded fp8 image;
                # psum is 32*qkv, pad stores PS*qkv -> drain scale 0.5
                for b in range(NBLK):
                    ps = ppool.tile([BLK, 8, WW], f32, name="ps1", tag="mm", bufs=3)
                    nc.tensor.matmul(ps[:].rearrange("p a c -> p (a c)"),
                                     w18t[:, b], xf8[:],
                                     start=True, stop=True,
                                     perf_mode=PM.DoubleRow)
                    dst = pad[b][:, 2 + 8 * nt:10 + 8 * nt, 2:2 + WW]
                    if b % 2 == 0:
                        nc.scalar.activation(out=dst, in_=ps[:], func=AF.Copy,
                                             bias=0.0, scale=PS / 32.0)
                    else:
                        nc.vector.tensor_scalar(out=dst, in0=ps[:],
                                                scalar1=PS / 32.0, scalar2=None,
                                                op0=ALU.mult)
                # pass 2: separated order (fp32r) -> Q + id-scale k/v stages
                ks, vs = [None, None], [None, None]
                for j in range(6):
                    ps = ppool.tile([128, TN], f32, name="ps2", tag="mm", bufs=3)
                    if j < 2:
                        for k in range(2):
                            sl_w = slice(128 * j, 128 * (j + 1))
                            nc.tensor.matmul(ps[:], w2qh[k][:, sl_w], xb[k][:],
                                             start=(k == 0), stop=False)
                            nc.tensor.matmul(ps[:], w2qh[k][:, sl_w], xl[k][:],
                                             start=False, stop=False)
                            nc.tensor.matmul(ps[:], w2ql[k][:, sl_w], xb[k][:],
                                             start=False, stop=(k == 1))
                    else:
                        for k in range(2):
                            nc.tensor.matmul(
                                ps[:], w2kv[k][:, 128 * (j - 2):128 * (j - 1)],
                                xb[k][:], start=(k == 0), stop=(k == 1))
                    if j < 2:
                        nc.scalar.activation(out=Q[j][:, TN * nt:TN * (nt + 1)], in_=ps[:],
                                             func=AF.Relu, bias=bi2[j][:], scale=1.0)
                    elif j < 4:
                        t = j - 2
                        kst = spool.tile([128, TN], bf16, name="ks", tag=f"ks{t}", bufs=4)
                        nc.scalar.activation(out=kst[:], in_=ps[:], func=AF.Relu,
                                             bias=bi2[j][:], scale=1.0)
                        ks[t] = kst
                    else:
                        t = j - 4
                        vst = spool.tile([128, TN], bf16, name="vs", tag=f"vs{t}", bufs=4)
                        nc.vector.tensor_scalar(out=vst[:], in0=ps[:], scalar1=bi2[j][:],
                                                scalar2=None, op0=ALU.add)
                        vs[t] = vst
                return ks, vs

            # ================= fused conv scales (fp8 DoubleRow) =============
            # tap pair u = row-major taps (2u, 2u+1); pair delta in the padded
            # image is off(2u+1)-off(2u); odd tail pairs (tap, tap) with
            # zeroed slot-1 weights (stride 0).
            def pair_deltas(taps):
                ds = []
                for u in range((len(taps) + 1) // 2):
                    if 2 * u + 1 < len(taps):
                        dy0, dx0 = taps[2 * u]
                        dy1, dx1 = taps[2 * u + 1]
                        ds.append((dy1 - dy0) * PADW + (dx1 - dx0))
                    else:
                        ds.append(0)
                return ds

            DELTAS = {3: pair_deltas(TAPS3), 5: pair_deltas(TAPS5)}

            def emit_conv_nt(nt):
                """Both conv scales for one spatial tile, all 8 blocks, then
                their attention stages."""
                stg = {}
                for s_idx in (1, 2):
                    for t in range(2):
                        stg[("k", s_idx, t)] = spool.tile(
                            [128, TN], bf16, name="ks", tag=f"ks{t}", bufs=4)
                        stg[("v", s_idx, t)] = spool.tile(
                            [128, TN], bf16, name="vs", tag=f"vs{t}", bufs=4)
                for b in range(NBLK):
                    for s, s_idx in ((3, 1), (5, 2)):
                        taps, U = TAPS3 if s == 3 else TAPS5, NU[s]
                        cp = ppool.tile([BLK, 8, WW], f32, name="cp", tag="conv",
                                        bufs=3)
                        # per image row: CoreSim's DoubleRow path needs the
                        # rhs to view as exactly [p, 2, N]
                        for u in range(U):
                            dy0, dx0 = taps[2 * u]
                            for r in range(8):
                                rhs = pad[b][:, 2 + 8 * nt + dy0 + r,
                                             2 + dx0:2 + dx0 + WW].copy()
                                rhs.ap.insert(1, [DELTAS[s][u], 2])
                                nc.tensor.matmul(cp[:, r], cw[s][b][:, u], rhs,
                                                 start=(u == 0 and r == 0),
                                                 stop=(u == U - 1 and r == 7),
                                                 perf_mode=PM.DoubleRow)
                        qt, qr = (256 * s_idx + 32 * b) // 128, (32 * b) % 128
                        t2, r2 = b // 4, (32 * b) % 128
                        # drains stay in psum scale (PS*WS); q/k scales cancel
                        # in the attention ratio, v scale folds into rcb
                        nc.scalar.activation(
                            out=Q[qt][qr:qr + 32, TN * nt:TN * (nt + 1)],
                            in_=cp[0:32].rearrange("p a c -> p (a c)"),
                            func=AF.Relu, bias=bc[s][b][0:32, :], scale=1.0)
                        nc.vector.tensor_scalar(
                            out=stg[("k", s_idx, t2)][r2:r2 + 32, :],
                            in0=cp[32:64].rearrange("p a c -> p (a c)"),
                            scalar1=bc[s][b][32:64, :], scalar2=0.0,
                            op0=ALU.add, op1=ALU.max)
                        if b % 2 == 0:
                            nc.scalar.activation(
                                out=stg[("v", s_idx, t2)][r2:r2 + 32, :],
                                in_=cp[64:96].rearrange("p a c -> p (a c)"),
                                func=AF.Identity, bias=bc[s][b][64:96, :],
                                scale=1.0)
                        else:
                            nc.vector.tensor_scalar(
                                out=stg[("v", s_idx, t2)][r2:r2 + 32, :],
                                in0=cp[64:96].rearrange("p a c -> p (a c)"),
                                scalar1=bc[s][b][64:96, :], scalar2=None,
                                op0=ALU.add)
                for s_idx in (1, 2):
                    process_stage(s_idx, nt,
                                  [stg[("k", s_idx, t)] for t in range(2)],
                                  [stg[("v", s_idx, t)] for t in range(2)])

            # Stream: pass12(nt) feeds pad rows; conv for nt-1 is ready once
            # pass 1 has written rows through nt (s5 needs dy<=+2).
            prev_stage = None
            for nt in range(NT):
                ksvs = emit_pass12(nt)
                if prev_stage is not None:
                    process_stage(0, nt - 1, *prev_stage)
                prev_stage = ksvs
                if nt >= 1:
                    emit_conv_nt(nt - 1)
            process_stage(0, NT - 1, *prev_stage)
            emit_conv_nt(NT - 1)

            # ====== assemble apply weights + denominator lhsT from vk ========
            appw = []
            denw = []
            for r in range(NREG):
                kf = qpool.tile([128, 1], f32, name=f"kfin_{r}")
                nc.vector.reduce_sum(out=kf[:], in_=kpart[r][:], axis=AX.X)
                dwt = qpool.tile([128, 96], bf16, name=f"denw_{r}")
                nc.gpsimd.tensor_scalar(out=dwt[:], in0=mdex[r][:],
                                        scalar1=kf[:], scalar2=None, op0=ALU.mult)
                denw.append(dwt)
                vks = vks_sb[r]
                aw = qpool.tile([128, 128], bf16, name=f"appw_{r}")
                for half in range(2):
                    nc.gpsimd.tensor_tensor(
                        out=aw[:, 64 * half:64 * (half + 1)].rearrange(
                            "p (d h) -> p d h", h=8),
                        in0=vks[:, 64 * half:64 * (half + 1)].rearrange(
                            "p (h d) -> p d h", d=8),
                        in1=mp[half][:].rearrange("p (d h) -> p d h", h=8),
                        op=ALU.mult)
                appw.append(aw)

            # ====== apply (pre-normalized q) + proj ==========================
            _pA.__exit__(None, None, None)
            _pB = tc.tile_pool(name="psumB", bufs=2, space="PSUM")
            ppb = _pB.__enter__()

            def emit_ddp(nt):
                """denominator dd[h, n] = den_h . q~_h(n) + eps, all 96 heads"""
                sl = slice(TN * nt, TN * (nt + 1))
                ddp = ppb.tile([96, TN], f32, name="ddp", tag="dd", bufs=2)
                nc.tensor.matmul(ddp[:], epsw[:], ones1[:], start=True, stop=False)
                for r in range(NREG):
                    nc.tensor.matmul(ddp[:], denw[r][:], Q[r][:, sl],
                                     start=False, stop=(r == NREG - 1))
                return ddp

            ddp_cur = emit_ddp(0)
            for nt in range(NT):
                sl = slice(TN * nt, TN * (nt + 1))
                rc = spool.tile([96, TN], f32, name="rc", tag="rc", bufs=1)
                scr = spool.tile([96, TN], f32, name="scr", tag="scr", bufs=1)
                nc.vector.reciprocal_approx_accurate(out=rc[:], in_=ddp_cur[:],
                                                     scratch=scr[:])
                rcb = spool.tile([96, TN], bf16, name="rcb", tag="rcb", bufs=1)
                # fold the per-head v-scale (1 id / SC conv) into the copy
                nc.scalar.activation(out=rcb[:], in_=rc[:], func=AF.Copy,
                                     bias=0.0, scale=svt[:])
                # hoist next tile's denominator matmuls to fill PE while the
                # reciprocal chain for this tile runs on DVE/ACT
                ddp_next = emit_ddp(nt + 1) if nt + 1 < NT else None
                att = []
                for r in range(NREG):
                    rcx = ppb.tile([128, TN], f32, name="rcx", tag="rcx", bufs=2)
                    nc.tensor.matmul(rcx[:], ert[r][:], rcb[:], start=True, stop=True)
                    rxb = spool.tile([128, TN], bf16, name="rxb", tag="rxb", bufs=3)
                    nc.scalar.copy(out=rxb[:], in_=rcx[:])
                    ap2 = ppb.tile([128, TN], f32, name="ap2", tag="ap2", bufs=2)
                    nc.tensor.matmul(ap2[:], appw[r][:], Q[r][:, sl],
                                     start=True, stop=True)
                    at = spool.tile([128, TN], bf16, name="at", tag="at", bufs=5)
                    nc.vector.tensor_tensor(out=at[:], in0=ap2[:], in1=rxb[:],
                                            op=ALU.mult)
                    att.append(at)
                for m in range(2):
                    pj = ppb.tile([128, TN], f32, name="pj", tag="pj", bufs=2)
                    for r in range(NREG):
                        nc.tensor.matmul(pj[:], pwt[r][:, 128 * m:128 * (m + 1)],
                                         att[r][:], start=(r == 0), stop=(r == NREG - 1))
                    ob = spool.tile([128, TN], f32, name="ob", tag="ob", bufs=2)
                    nc.scalar.activation(out=ob[:], in_=pj[:], func=AF.Identity,
                                         bias=pbt[m][:], scale=1.0)
                    nc.sync.dma_start(
                        out=d_out[128 * m:128 * (m + 1), TN * nt:TN * (nt + 1)], in_=ob[:])
                ddp_cur = ddp_next
            _pB.__exit__(None, None, None)
    return nc


def _get_nc():
    if "nc" not in _cache:
        nc = _build()
        nc.compile()
        _cache["nc"] = nc
    return _cache["nc"]


def _feeds(inputs):
    import ml_dtypes

    def bf(a):
        return np.asarray(a, np.float32).astype(ml_dtypes.bfloat16)

    d = _host_weights(inputs)
    base = {
        "w2qh": bf(np.ascontiguousarray(d["w2t"][:, :256])),
        "w2ql": bf(np.ascontiguousarray(d["w2t"][:, :256])
                   - np.asarray(bf(np.ascontiguousarray(d["w2t"][:, :256])),
                                np.float32)),
        "bi2": d["bi2"].astype(np.float32),
        "fw3": d["fw3"].astype(ml_dtypes.float8_e4m3),
        "fw5": d["fw5"].astype(ml_dtypes.float8_e4m3),
        "bc3": d["bc3"].astype(np.float32), "bc5": d["bc5"].astype(np.float32),
        "idt": bf(d["idt"]),
        "mp0": bf(d["mp0"]), "mp1": bf(d["mp1"]),
        "er": bf(d["er"]), "mdex": bf(d["mdex"]),
        "pw2": bf(d["pw2"]), "pb": d["pb"].astype(np.float32),
        "svec": d["svec"].astype(np.float32),
        "w18": d["w18"].astype(ml_dtypes.float8_e4m3),
        "w2kv": bf(np.ascontiguousarray(d["w2t"][:, 256:])),
    }
    x = np.asarray(inputs["x"], np.float32).reshape(B, CIN, N)
    return base, x


def kernel(**inputs):
    from concourse.bass_utils import run_bass_kernel_spmd

    base, x = _feeds(inputs)
    in_maps = []
    for c in range(B):
        m = dict(base)
        m["xf"] = np.ascontiguousarray(x[c])
        in_maps.append(m)
    nc = _get_nc()
    res = run_bass_kernel_spmd(nc, in_maps, list(range(B))).results
    out = np.stack([np.asarray(r["out"]).reshape(CIN, HH, WW) for r in res])
    return out.astype(np.float32)



# revision 63
# speedup vs baseline: 1.0615x; 1.0615x over previous
"""LiteMLA (EfficientViT multi-scale linear attention) Trainium2 Bass kernel.

Sharding: data-parallel over batch B=8 across 8 NeuronCores (1 image/core).
Per-core pipeline:
  1. Streamed per-nt x: pass1 (bf16, natural channel order) -> zero-padded
     SBUF image for conv taps; pass2 -> attention Q buffer + id-scale K/V
     stages. The q channels use Dekker-split bf16 matmuls (whi.xhi +
     whi.xlo + wlo.xhi, ~16-bit effective mantissa): the id-scale heads are
     ill-conditioned (att = (vk@relu q)/(den@relu q) is 0/0 at positions
     where all 8 q dims are negative), so relu(q)'s sign pattern must track
     the fp32 reference closely; plain bf16 or HW-float32r inputs flip
     signs and cost ~0.15 rel err. k/v stay bf16 (4096-term averages).
     s3 conv is emitted interleaved into this loop to fill PE idle.
  2. s3/s5: depthwise 3x3/5x5 + grouped 1x1 FUSED on host into per-tap
     block-diagonal [96,96] weights; all taps of a block fetched in ONE
     sync-queue DMA; PE matmuls accumulate taps in PSUM reading shifted
     slices of the padded image.
  3. relu-linear attention: per spatial tile, relu(k)/v transposed by the
     DMA engines (xbar dma_start_transpose, contiguous [128,128] dst tiles)
     and reduced into per-16-head vk outer products; per-tile PSUM partials
     fold into SBUF accumulators on DVE; denominator = row-sums of relu(k).
  4. apply: att_raw = vk @ relu(q) via block-diagonal apply weights on raw
     Q (no per-position pre-scale, keeping PE free of the normalize chain);
     the batched [96,TN] denominator reciprocal is broadcast-expanded per
     region on PE and applied at the PSUM drain (one DVE multiply); proj
     contracts 128 rows (2 groups per matmul). Each tile's attention stage
     is processed one iteration deferred so its DMA-transpose issues overlap
     the next tile's matmuls.

All SBUF/PSUM operand slices start at partition 0/32/64/96 (HW requirement).
"""

import sys
import numpy as np

sys.path.insert(0, "/opt/trn_rl_repo")

B, CIN, HH, WW = 8, 256, 64, 64
N = HH * WW            # 4096
HEADS = 32             # per scale
C3 = 768
NHEADS = 96
PADW = WW + 4          # 68
NT = 8                 # spatial tiles of 512 positions (8 image rows each)
TN = 512
HALF = 2               # nts processed per conv weight fetch
TAPS3 = [(dy, dx) for dy in (-1, 0, 1) for dx in (-1, 0, 1)]
TAPS5 = [(dy, dx) for dy in (-2, -1, 0, 1, 2) for dx in (-2, -1, 0, 1, 2)]
NBLK = 8               # conv channel blocks of 4 head-groups
BLK = 96
NREG = 6               # vk regions of 16 heads

_cache = {}

PS = 16.0              # fp8 pad image pre-scale
WS = 256.0             # fp8 fused conv weight pre-scale
SC = 1.0 / (PS * WS)   # conv psum drain scale (2^-12)
NU = {3: 5, 5: 13}     # DoubleRow tap-pair units per scale


def _head_of(g12, i):
    return 16 * (g12 // 2) + 8 * (g12 % 2) + i


def _host_weights(inp):
    f32 = np.float32
    W = np.asarray(inp["qkv_w"], f32)[:, :, 0, 0]            # [768, 256]
    qkv_b = np.asarray(inp["qkv_b"], f32)
    pw = {3: np.asarray(inp["pw3_w"], f32)[:, :, 0, 0],
          5: np.asarray(inp["pw5_w"], f32)[:, :, 0, 0]}
    pwb = {3: np.asarray(inp["pw3_b"], f32), 5: np.asarray(inp["pw5_b"], f32)}
    dw = {3: np.asarray(inp["dw3_w"], f32)[:, 0],
          5: np.asarray(inp["dw5_w"], f32)[:, 0]}
    dwb = {3: np.asarray(inp["dw3_b"], f32), 5: np.asarray(inp["dw5_b"], f32)}
    proj_w = np.asarray(inp["proj_w"], f32)[:, :, 0, 0]      # [256, 768]
    proj_b = np.asarray(inp["proj_b"], f32)

    d = {}
    # pass1 weights pre-scaled by PS: pad fp8 image = PS*qkv via a pure-copy
    # drain (no scale op needed)
    d["w1t"] = np.ascontiguousarray(W.T) * PS                # [256, 768]
    # per-head reciprocal scale: attention is scale-invariant in q/k per head
    # and scales linearly with v, so conv drains skip the 1/(PS*WS) descale
    # and the v-scale is folded into the rcb copy (heads 32..95 are conv)
    sv = np.ones((NHEADS, 1), f32)
    sv[32:] = 1.0 / (PS * WS)
    # conv att pre-scaled 512x so its fp8 att tiles (DoubleRow proj) stay in
    # e4m3 normal range; id att stays bf16 (dominates output magnitude)
    sv[32:] *= 512.0
    d["svec"] = sv
    # pass1 fp8 DoubleRow weights: [NBLK, 128, 2, 96], *32 (drain scale .5*PS/16)
    w18 = np.empty((NBLK, 128, 2, BLK), f32)
    for b in range(NBLK):
        for j in range(2):
            w18[b, :, j, :] = 32.0 * W[BLK * b:BLK * (b + 1), 128 * j:128 * (j + 1)].T
    d["w18"] = w18
    perm2 = np.empty(768, np.int64)
    for h in range(HEADS):
        for e in range(8):
            perm2[h * 8 + e] = h * 24 + e
            perm2[256 + h * 8 + e] = h * 24 + 8 + e
            perm2[512 + h * 8 + e] = h * 24 + 16 + e
    d["w2t"] = np.ascontiguousarray(W[perm2].T)
    d["bi2"] = qkv_b[perm2].reshape(768, 1)

    # fused conv weights: per tap, 8 blocks of 4 groups, [96in, 96out q|k|v];
    # stored per-block contiguous over taps: [NBLK, 96in, taps*96out]
    for s, taps in ((3, TAPS3), (5, TAPS5)):
        fw = np.zeros((len(taps), NBLK, BLK, BLK), f32)
        fb = np.zeros((NBLK, BLK), f32)
        for b in range(NBLK):
            for gl in range(4):
                g = 4 * b + gl
                M24 = pw[s][g * 24:(g + 1) * 24]             # [24 out(oo), 24 in]
                bias24 = pwb[s][g * 24:(g + 1) * 24] + M24 @ dwb[s][g * 24:(g + 1) * 24]
                for oo in range(24):
                    m = (oo // 8) * 32 + gl * 8 + (oo % 8)   # [q32|k32|v32]
                    fb[b, m] = bias24[oo]
                dvec = dw[s][g * 24:(g + 1) * 24]            # [24 in, kh, kw]
                for ti, (dy, dx) in enumerate(taps):
                    wt = M24 * dvec[:, dy + s // 2, dx + s // 2][None, :]
                    for oo in range(24):
                        m = (oo // 8) * 32 + gl * 8 + (oo % 8)
                        fw[ti, b, gl * 24:(gl + 1) * 24, m] = wt[oo]
        # fp8 DoubleRow pair-stacked: [NBLK, BLK_in, U, 2, BLK_out]; unit u
        # holds taps (2u, 2u+1); odd tap count -> last unit slot 1 zeroed
        U = NU[s]
        fw8 = np.zeros((NBLK, BLK, U, 2, BLK), f32)
        for u in range(U):
            for j in range(2):
                ti = 2 * u + j
                if ti < len(taps):
                    fw8[:, :, u, j, :] = fw[ti] * WS
        d[f"fw{s}"] = fw8
        # conv drain bias in raw-psum units (outputs stay scaled by PS*WS)
        d[f"bc{s}"] = fb.reshape(NBLK, BLK, 1) * (PS * WS)

    d["idt"] = np.eye(128, dtype=f32)

    # masks for vk -> apply-weight assembly (dd-major cols, no den col)
    for half in range(2):
        mp = np.zeros((128, 64), f32)   # [(hp,e), (dd,h)]
        for p in range(128):
            hp = p // 8
            for h in range(8):
                if hp == h + 8 * half:
                    for dd in range(8):
                        mp[p, 8 * dd + h] = 1.0
        d[f"mp{half}"] = mp

    # expand matrices: er maps rc rows (96 heads) onto ap2's row layout
    # (col p: half=p//64, dd, i -> head 16r+8*(p//64)+p%8); mdex places den
    # values for the Q-region row layout (row p -> head 16r + p//8)
    er = np.zeros((NREG, 96, 128), f32)
    mdex = np.zeros((NREG, 128, 96), f32)
    for r in range(NREG):
        for p in range(128):
            er[r, 16 * r + 8 * (p // 64) + p % 8, p] = 1.0
            mdex[r, p, 16 * r + p // 8] = 1.0
    d["er"] = er
    d["mdex"] = mdex

    # proj lhsT [6, 128, 256]: rows 0:64 group 2r, 64:128 group 2r+1; row
    # (64*half + 8*dd + i) = proj col of head _head_of(2r+half, i), dim dd
    PW2 = np.zeros((NREG, 128, 256), f32)
    for r in range(NREG):
        for half in range(2):
            for i in range(8):
                Hh = _head_of(2 * r + half, i)
                for dd in range(8):
                    PW2[r, 64 * half + 8 * dd + i] = proj_w[:, 8 * Hh + dd]
    # proj: id regions 0,1 in bf16 at psum scale WP; conv region pairs
    # (2,3),(4,5) as fp8 DoubleRow k-tiles at WP/512 (att carries the 512)
    WP = 2048.0
    d["pwid"] = WP * PW2[0:2]
    pw8 = np.empty((2, 128, 2, 256), f32)
    for p in range(2):
        for j in range(2):
            pw8[p, :, j, :] = (WP / 512.0) * PW2[2 + 2 * p + j]
    d["pw8"] = pw8
    d["pb"] = proj_b.reshape(256, 1)
    return d


def _build():
    import concourse.bass as bass
    import concourse.bacc as bacc_mod
    import concourse.mybir as mybir
    from concourse.tile import TileContext

    dt = mybir.dt
    f32, bf16, f8 = dt.float32, dt.bfloat16, dt.float8e4
    AF = mybir.ActivationFunctionType
    ALU = mybir.AluOpType
    AX = mybir.AxisListType
    PM = mybir.MatmulPerfMode

    nc = bacc_mod.Bacc()
    x_in = nc.dram_tensor("xf", [CIN, N], f32, kind="ExternalInput")
    dW2QH = nc.dram_tensor("w2qh", [CIN, 256], bf16, kind="ExternalInput")
    dW2QL = nc.dram_tensor("w2ql", [CIN, 256], bf16, kind="ExternalInput")
    dBI2 = nc.dram_tensor("bi2", [C3, 1], f32, kind="ExternalInput")
    dFW3 = nc.dram_tensor("fw3", [NBLK, BLK, NU[3], 2, BLK], f8, kind="ExternalInput")
    dFW5 = nc.dram_tensor("fw5", [NBLK, BLK, NU[5], 2, BLK], f8, kind="ExternalInput")
    dBC3 = nc.dram_tensor("bc3", [NBLK, BLK, 1], f32, kind="ExternalInput")
    dBC5 = nc.dram_tensor("bc5", [NBLK, BLK, 1], f32, kind="ExternalInput")
    dIDT = nc.dram_tensor("idt", [128, 128], bf16, kind="ExternalInput")
    dMP = [nc.dram_tensor(f"mp{h}", [128, 64], bf16, kind="ExternalInput") for h in range(2)]
    dER = nc.dram_tensor("er", [NREG, 96, 128], bf16, kind="ExternalInput")
    dMDEX = nc.dram_tensor("mdex", [NREG, 128, 96], bf16, kind="ExternalInput")
    dPW8 = nc.dram_tensor("pw8", [2, 128, 2, 256], f8, kind="ExternalInput")
    dPWI = nc.dram_tensor("pwid", [2, 128, 256], bf16, kind="ExternalInput")
    dPB = nc.dram_tensor("pb", [256, 1], f32, kind="ExternalInput")
    dSV = nc.dram_tensor("svec", [NHEADS, 1], f32, kind="ExternalInput")
    dW18 = nc.dram_tensor("w18", [NBLK, 128, 2, BLK], f8, kind="ExternalInput")
    dW2kv = nc.dram_tensor("w2kv", [CIN, 512], bf16, kind="ExternalInput")
    d_out = nc.dram_tensor("out", [CIN, N], f32, kind="ExternalOutput")

    with TileContext(nc) as tc:
        with (
            tc.tile_pool(name="consts", bufs=1) as cpool,
            tc.tile_pool(name="persist", bufs=1) as qpool,
            tc.tile_pool(name="wstream", bufs=2) as wpool,
            tc.tile_pool(name="stage", bufs=2) as spool,
        ):
            # ---- constants (off the sync queue so x DMAs start immediately) --
            w2qh = [cpool.tile([128, 256], bf16, name=f"w2qh_{k}") for k in range(2)]
            w2ql = [cpool.tile([128, 256], bf16, name=f"w2ql_{k}") for k in range(2)]
            for k in range(2):
                nc.scalar.dma_start(out=w2qh[k][:], in_=dW2QH[128 * k:128 * (k + 1), :])
                nc.scalar.dma_start(out=w2ql[k][:], in_=dW2QL[128 * k:128 * (k + 1), :])
            bi2 = [cpool.tile([128, 1], f32, name=f"bi2_{j}") for j in range(6)]
            for j in range(6):
                nc.scalar.dma_start(out=bi2[j][:], in_=dBI2[128 * j:128 * (j + 1), :])
            # pass-1/2 weights FIRST: they gate the very first PE work;
            # the bulky conv weights follow (not needed until conv(0) ~15us in)
            w18t = cpool.tile([128, NBLK, 2, BLK], f8, name="w18t")
            for b in range(NBLK):
                nc.scalar.dma_start(out=w18t[:, b], in_=dW18[b])
            w2kv = [cpool.tile([128, 512], bf16, name=f"w2kv_{k}") for k in range(2)]
            for k in range(2):
                nc.scalar.dma_start(out=w2kv[k][:], in_=dW2kv[128 * k:128 * (k + 1), :])
            bc = {}
            for s, db in ((3, dBC3), (5, dBC5)):
                bc[s] = [cpool.tile([BLK, 1], f32, name=f"bc{s}_{b}") for b in range(NBLK)]
                for b in range(NBLK):
                    nc.gpsimd.dma_start(out=bc[s][b][:], in_=db[b])
            # resident fp8 DoubleRow conv weights: [96, U, 2, 96] per block
            cw = {}
            for s, dfw in ((3, dFW3), (5, dFW5)):
                cw[s] = [cpool.tile([BLK, NU[s], 2, BLK], f8, name=f"cw{s}_{b}")
                         for b in range(NBLK)]
                for b in range(NBLK):
                    eng = nc.scalar if b % 2 == 0 else nc.gpsimd
                    eng.dma_start(out=cw[s][b][:], in_=dfw[b])
            idt = cpool.tile([128, 128], bf16, name="idt")
            nc.scalar.dma_start(out=idt[:], in_=dIDT[:, :])
            mp = [cpool.tile([128, 64], bf16, name=f"mp_{h}") for h in range(2)]
            for h in range(2):
                nc.gpsimd.dma_start(out=mp[h][:], in_=dMP[h][:, :])
            ert = [cpool.tile([96, 128], bf16, name=f"er_{r}") for r in range(NREG)]
            mdex = [cpool.tile([128, 96], bf16, name=f"mdex_{r}") for r in range(NREG)]
            pw8t = [cpool.tile([128, 2, 256], f8, name=f"pw8_{p}") for p in range(2)]
            pwid = [cpool.tile([128, 256], bf16, name=f"pwid_{r}") for r in range(2)]
            for p in range(2):
                nc.gpsimd.dma_start(out=pw8t[p][:], in_=dPW8[p])
                nc.gpsimd.dma_start(out=pwid[p][:], in_=dPWI[p])
            for r in range(NREG):
                nc.gpsimd.dma_start(out=ert[r][:], in_=dER[r])
                nc.gpsimd.dma_start(out=mdex[r][:], in_=dMDEX[r])
            pbt = [cpool.tile([128, 1], f32, name=f"pbt_{m}") for m in range(2)]
            for m in range(2):
                nc.gpsimd.dma_start(out=pbt[m][:], in_=dPB[128 * m:128 * (m + 1), :])
            svt = cpool.tile([NHEADS, 1], f32, name="svt")
            nc.gpsimd.dma_start(out=svt[:], in_=dSV[:, :])
            epsw = cpool.tile([1, 96], bf16, name="epsw")
            nc.gpsimd.memset(epsw[:], 1e-15)
            ones1 = cpool.tile([1, TN], bf16, name="ones1")
            nc.gpsimd.memset(ones1[:], 1.0)

            # ---- persistent activations ----
            pad = [qpool.tile([BLK, PADW, PADW], f8, name=f"pad_{b}") for b in range(NBLK)]
            for b in range(NBLK):
                # zero only the 2-wide borders; interior is fully written by
                # pass 1 (keeps these memsets off pass 1's dependency chain)
                nc.gpsimd.memset(pad[b][:, 0:2, :], 0.0)
                nc.gpsimd.memset(pad[b][:, PADW - 2:PADW, :], 0.0)
                nc.gpsimd.memset(pad[b][:, 2:PADW - 2, 0:2], 0.0)
                nc.gpsimd.memset(pad[b][:, 2:PADW - 2, PADW - 2:PADW], 0.0)
            Q = [qpool.tile([128, N], bf16, name=f"Q_{r}") for r in range(NREG)]
            kpart = [qpool.tile([128, NT], f32, name=f"kpart_{r}") for r in range(NREG)]
            vks_sb = [qpool.tile([128, 128], bf16, name=f"vks_{r}") for r in range(NREG)]

            vks_acc = [qpool.tile([128, 128], f32, name=f"vka_{r}")
                       for r in range(NREG)]

            # PSUM pool A: phases 1 + conv (reclaimed before apply phase)
            _pA = tc.tile_pool(name="psumA", bufs=2, space="PSUM")
            ppool = _pA.__enter__()

            # ============ shared per-tile attention stage ====================
            def process_stage(s_idx, nt, ks, vs):
                # first two id-stages: PE transposes (PE is idle pre-conv and
                # this keeps DMA-transfer latency off the early critical path)
                pe_tr = False
                """ks/vs: 2 bf16 [128,512] stage tiles (relu'd k / raw v)."""
                for t in range(2):
                    r = 2 * s_idx + t
                    nc.vector.reduce_sum(out=kpart[r][:, nt:nt + 1], in_=ks[t][:], axis=AX.X)
                for jj in range(4):
                    for t in range(2):
                        kT = spool.tile([128, 128], bf16, name="kT", tag="kT", bufs=12)
                        vT = spool.tile([128, 128], bf16, name="vT", tag="vT", bufs=12)
                        if pe_tr:
                            tp = ppool.tile([128, 128], bf16, name="tp", tag="mm",
                                            bufs=2)
                            nc.tensor.transpose(
                                tp[:], ks[t][:, 128 * jj:128 * (jj + 1)], idt[:])
                            nc.scalar.copy(out=kT[:], in_=tp[:])
                            tp2 = ppool.tile([128, 128], bf16, name="tp2", tag="mm",
                                             bufs=2)
                            nc.tensor.transpose(
                                tp2[:], vs[t][:, 128 * jj:128 * (jj + 1)], idt[:])
                            nc.vector.tensor_copy(out=vT[:], in_=tp2[:])
                        else:
                            nc.sync.dma_start_transpose(
                                out=kT[:], in_=ks[t][:, 128 * jj:128 * (jj + 1)])
                            nc.sync.dma_start_transpose(
                                out=vT[:], in_=vs[t][:, 128 * jj:128 * (jj + 1)])
                        nc.tensor.matmul(vkps[t][:], kT[:], vT[:],
                                         start=(jj == 0), stop=(jj == 3))
                for t in range(2):
                    r = 2 * s_idx + t
                    if nt == 0:
                        nc.vector.tensor_copy(out=vks_acc[r][:], in_=vkps[t][:])
                    else:
                        nc.vector.tensor_tensor(out=vks_acc[r][:], in0=vks_acc[r][:],
                                                in1=vkps[t][:], op=ALU.add)
                    if nt == NT - 1:
                        nc.gpsimd.tensor_copy(out=vks_sb[r][:], in_=vks_acc[r][:])

            # ===== pass 1+2 per nt: stream x, build pad image + id stage =====

            def emit_pass12(nt):
                xt = [spool.tile([128, TN], f32, name="xt", tag=f"xt{k}", bufs=2)
                      for k in range(2)]
                xb = [spool.tile([128, TN], bf16, name="xb", tag=f"xb{k}", bufs=2)
                      for k in range(2)]
                xl = [spool.tile([128, TN], bf16, name="xl", tag=f"xl{k}", bufs=2)
                      for k in range(2)]
                xf8 = spool.tile([128, 2, TN], f8, name="xf8", tag="xf8", bufs=2)
                for k in range(2):
                    nc.sync.dma_start(out=xt[k][:],
                                      in_=x_in[128 * k:128 * (k + 1), TN * nt:TN * (nt + 1)])
                    nc.gpsimd.tensor_copy(out=xb[k][:], in_=xt[k][:])
                    nc.gpsimd.tensor_tensor(out=xl[k][:], in0=xt[k][:], in1=xb[k][:],
                                            op=ALU.subtract)
                    nc.gpsimd.tensor_copy(out=xf8[:, k, :], in_=xt[k][:])
                # pass 1 (fp8 DoubleRow): natural order -> padded fp8 image;
                # psum is 32*qkv, pad stores PS*qkv -> drain scale 0.5
                for b in range(NBLK):
                    ps = ppool.tile([BLK, 8, WW], f32, name="ps1", tag="mm", bufs=2)
                    nc.tensor.matmul(ps[:].rearrange("p a c -> p (a c)"),
                                     w18t[:, b], xf8[:],
                                     start=True, stop=True,
                                     perf_mode=PM.DoubleRow)
                    dst = pad[b][:, 2 + 8 * nt:10 + 8 * nt, 2:2 + WW]
                    if b % 2 == 0:
                        nc.scalar.activation(out=dst, in_=ps[:], func=AF.Copy,
                                             bias=0.0, scale=PS / 32.0)
                    else:
                        nc.vector.tensor_scalar(out=dst, in0=ps[:],
                                                scalar1=PS / 32.0, scalar2=None,
                                                op0=ALU.mult)
                # pass 2: separated order (fp32r) -> Q + id-scale k/v stages
                ks, vs = [None, None], [None, None]
                for j in range(6):
                    ps = ppool.tile([128, TN], f32, name="ps2", tag="mm", bufs=2)
                    if j < 2:
                        for k in range(2):
                            sl_w = slice(128 * j, 128 * (j + 1))
                            nc.tensor.matmul(ps[:], w2qh[k][:, sl_w], xb[k][:],
                                             start=(k == 0), stop=False)
                            nc.tensor.matmul(ps[:], w2qh[k][:, sl_w], xl[k][:],
                                             start=False, stop=False)
                            nc.tensor.matmul(ps[:], w2ql[k][:, sl_w], xb[k][:],
                                             start=False, stop=(k == 1))
                    else:
                        for k in range(2):
                            nc.tensor.matmul(
                                ps[:], w2kv[k][:, 128 * (j - 2):128 * (j - 1)],
                                xb[k][:], start=(k == 0), stop=(k == 1))
                    if j < 2:
                        nc.scalar.activation(out=Q[j][:, TN * nt:TN * (nt + 1)], in_=ps[:],
                                             func=AF.Relu, bias=bi2[j][:], scale=1.0)
                    elif j < 4:
                        t = j - 2
                        kst = spool.tile([128, TN], bf16, name="ks", tag=f"ks{t}", bufs=4)
                        nc.scalar.activation(out=kst[:], in_=ps[:], func=AF.Relu,
                                             bias=bi2[j][:], scale=1.0)
                        ks[t] = kst
                    else:
                        t = j - 4
                        vst = spool.tile([128, TN], bf16, name="vs", tag=f"vs{t}", bufs=4)
                        nc.vector.tensor_scalar(out=vst[:], in0=ps[:], scalar1=bi2[j][:],
                                                scalar2=None, op0=ALU.add)
                        vs[t] = vst
                return ks, vs

            # ================= fused conv scales (fp8 DoubleRow) =============
            # tap pair u = row-major taps (2u, 2u+1); pair delta in the padded
            # image is off(2u+1)-off(2u); odd tail pairs (tap, tap) with
            # zeroed slot-1 weights (stride 0).
            def pair_deltas(taps):
                ds = []
                for u in range((len(taps) + 1) // 2):
                    if 2 * u + 1 < len(taps):
                        dy0, dx0 = taps[2 * u]
                        dy1, dx1 = taps[2 * u + 1]
                        ds.append((dy1 - dy0) * PADW + (dx1 - dx0))
                    else:
                        ds.append(0)
                return ds

            DELTAS = {3: pair_deltas(TAPS3), 5: pair_deltas(TAPS5)}

            def emit_conv_nt(nt):
                """Both conv scales for one spatial tile, all 8 blocks, then
                their attention stages."""
                stg = {}
                for s_idx in (1, 2):
                    for t in range(2):
                        stg[("k", s_idx, t)] = spool.tile(
                            [128, TN], bf16, name="ks", tag=f"ks{t}", bufs=4)
                        stg[("v", s_idx, t)] = spool.tile(
                            [128, TN], bf16, name="vs", tag=f"vs{t}", bufs=4)
                for b in range(NBLK):
                    for s, s_idx in ((3, 1), (5, 2)):
                        taps, U = TAPS3 if s == 3 else TAPS5, NU[s]
                        cp = ppool.tile([BLK, 8, WW], f32, name="cp", tag="conv",
                                        bufs=4)
                        # per image row: CoreSim's DoubleRow path needs the
                        # rhs to view as exactly [p, 2, N]
                        for u in range(U):
                            dy0, dx0 = taps[2 * u]
                            for r in range(8):
                                rhs = pad[b][:, 2 + 8 * nt + dy0 + r,
                                             2 + dx0:2 + dx0 + WW].copy()
                                rhs.ap.insert(1, [DELTAS[s][u], 2])
                                nc.tensor.matmul(cp[:, r], cw[s][b][:, u], rhs,
                                                 start=(u == 0 and r == 0),
                                                 stop=(u == U - 1 and r == 7),
                                                 perf_mode=PM.DoubleRow)
                        qt, qr = (256 * s_idx + 32 * b) // 128, (32 * b) % 128
                        t2, r2 = b // 4, (32 * b) % 128
                        # drains stay in psum scale (PS*WS); q/k scales cancel
                        # in the attention ratio, v scale folds into rcb
                        nc.scalar.activation(
                            out=Q[qt][qr:qr + 32, TN * nt:TN * (nt + 1)],
                            in_=cp[0:32].rearrange("p a c -> p (a c)"),
                            func=AF.Relu, bias=bc[s][b][0:32, :], scale=1.0)
                        if b >= 7:
                            nc.scalar.activation(
                                out=stg[("k", s_idx, t2)][r2:r2 + 32, :],
                                in_=cp[32:64].rearrange("p a c -> p (a c)"),
                                func=AF.Relu, bias=bc[s][b][32:64, :], scale=1.0)
                        else:
                            nc.vector.tensor_scalar(
                                out=stg[("k", s_idx, t2)][r2:r2 + 32, :],
                                in0=cp[32:64].rearrange("p a c -> p (a c)"),
                                scalar1=bc[s][b][32:64, :], scalar2=0.0,
                                op0=ALU.add, op1=ALU.max)
                        if b % 2 == 0:
                            nc.scalar.activation(
                                out=stg[("v", s_idx, t2)][r2:r2 + 32, :],
                                in_=cp[64:96].rearrange("p a c -> p (a c)"),
                                func=AF.Identity, bias=bc[s][b][64:96, :],
                                scale=1.0)
                        else:
                            nc.vector.tensor_scalar(
                                out=stg[("v", s_idx, t2)][r2:r2 + 32, :],
                                in0=cp[64:96].rearrange("p a c -> p (a c)"),
                                scalar1=bc[s][b][64:96, :], scalar2=None,
                                op0=ALU.add)
                for s_idx in (1, 2):
                    process_stage(s_idx, nt,
                                  [stg[("k", s_idx, t)] for t in range(2)],
                                  [stg[("v", s_idx, t)] for t in range(2)])

            # Stream: pass12(nt) feeds pad rows; conv for nt-1 is ready once
            # pass 1 has written rows through nt (s5 needs dy<=+2).
            prev_stage = None
            for nt in range(NT):
                ksvs = emit_pass12(nt)
                if prev_stage is not None:
                    process_stage(0, nt - 1, *prev_stage)
                prev_stage = ksvs
                if nt >= 1:
                    emit_conv_nt(nt - 1)
            process_stage(0, NT - 1, *prev_stage)
            emit_conv_nt(NT - 1)

            # ====== assemble apply weights + denominator lhsT from vk ========
            appw = []
            denw = []
            for r in range(NREG):
                kf = qpool.tile([128, 1], f32, name=f"kfin_{r}")
                nc.vector.reduce_sum(out=kf[:], in_=kpart[r][:], axis=AX.X)
                dwt = qpool.tile([128, 96], bf16, name=f"denw_{r}")
                nc.gpsimd.tensor_scalar(out=dwt[:], in0=mdex[r][:],
                                        scalar1=kf[:], scalar2=None, op0=ALU.mult)
                denw.append(dwt)
                vks = vks_sb[r]
                aw = qpool.tile([128, 128], bf16, name=f"appw_{r}")
                for half in range(2):
                    nc.gpsimd.tensor_tensor(
                        out=aw[:, 64 * half:64 * (half + 1)].rearrange(
                            "p (d h) -> p d h", h=8),
                        in0=vks[:, 64 * half:64 * (half + 1)].rearrange(
                            "p (h d) -> p d h", d=8),
                        in1=mp[half][:].rearrange("p (d h) -> p d h", h=8),
                        op=ALU.mult)
                appw.append(aw)

            # ====== apply (pre-normalized q) + proj ==========================
            _pA.__exit__(None, None, None)
            _pd.__exit__(None, None, None)
            _pB = tc.tile_pool(name="psumB", bufs=2, space="PSUM")
            ppb = _pB.__enter__()

            def emit_ddp(nt):
                """denominator dd[h, n] = den_h . q~_h(n) + eps, all 96 heads"""
                sl = slice(TN * nt, TN * (nt + 1))
                ddp = ppb.tile([96, TN], f32, name="ddp", tag="dd", bufs=2)
                nc.tensor.matmul(ddp[:], epsw[:], ones1[:], start=True, stop=False)
                for r in range(NREG):
                    nc.tensor.matmul(ddp[:], denw[r][:], Q[r][:, sl],
                                     start=False, stop=(r == NREG - 1))
                return ddp

            ddp_cur = emit_ddp(0)
            for nt in range(NT):
                sl = slice(TN * nt, TN * (nt + 1))
                rc = spool.tile([96, TN], f32, name="rc", tag="rc", bufs=1)
                nc.vector.reciprocal_approx_fast(out=rc[:], in_=ddp_cur[:])
                rcb = spool.tile([96, TN], bf16, name="rcb", tag="rcb", bufs=1)
                # fold the per-head v-scale (1 id / SC conv) into the copy
                nc.scalar.activation(out=rcb[:], in_=rc[:], func=AF.Copy,
                                     bias=0.0, scale=svt[:])
                atid = []
                at2 = [None, None]
                for r in range(NREG):
                    rcx = ppb.tile([128, TN], f32, name="rcx", tag="rcx", bufs=2)
                    nc.tensor.matmul(rcx[:], ert[r][:], rcb[:], start=True, stop=True)
                    rxb = spool.tile([128, TN], bf16, name="rxb", tag="rxb", bufs=6)
                    nc.scalar.copy(out=rxb[:], in_=rcx[:])
                    ap2 = ppb.tile([128, TN], f32, name="ap2", tag="ap2", bufs=2)
                    nc.tensor.matmul(ap2[:], appw[r][:], Q[r][:, sl],
                                     start=True, stop=True)
                    if r < 2:
                        at = spool.tile([128, TN], bf16, name="at", tag="at",
                                        bufs=3)
                        nc.vector.tensor_tensor(out=at[:], in0=ap2[:],
                                                in1=rxb[:], op=ALU.mult)
                        atid.append(at)
                    else:
                        p = (r - 2) // 2
                        if r % 2 == 0:
                            at2[p] = spool.tile([128, 2, TN], f8, name="at2",
                                                tag="at2", bufs=3)
                        nc.vector.tensor_tensor(out=at2[p][:, r % 2], in0=ap2[:],
                                                in1=rxb[:], op=ALU.mult)
                # hoist next tile's denominator matmuls here so the at-mults
                # (DVE) finish while PE runs them; pj then starts unstalled
                ddp_next = emit_ddp(nt + 1) if nt + 1 < NT else None
                for m in range(2):
                    pj = ppb.tile([128, TN], f32, name="pj", tag="pj", bufs=2)
                    for r in range(2):
                        nc.tensor.matmul(pj[:], pwid[r][:, 128 * m:128 * (m + 1)],
                                         atid[r][:], start=(r == 0), stop=False)
                    for p in range(2):
                        nc.tensor.matmul(pj[:], pw8t[p][:, :, 128 * m:128 * (m + 1)],
                                         at2[p][:], start=False, stop=(p == 1),
                                         perf_mode=PM.DoubleRow)
                    ob = spool.tile([128, TN], f32, name="ob", tag="ob", bufs=2)
                    nc.scalar.activation(out=ob[:], in_=pj[:], func=AF.Identity,
                                         bias=pbt[m][:], scale=1.0 / 2048.0)
                    nc.sync.dma_start(
                        out=d_out[128 * m:128 * (m + 1), TN * nt:TN * (nt + 1)], in_=ob[:])
                ddp_cur = ddp_next
            _pB.__exit__(None, None, None)
    return nc


def _get_nc():
    if "nc" not in _cache:
        nc = _build()
        nc.compile()
        _cache["nc"] = nc
    return _cache["nc"]


def _feeds(inputs):
    import ml_dtypes

    def bf(a):
        return np.asarray(a, np.float32).astype(ml_dtypes.bfloat16)

    d = _host_weights(inputs)
    base = {
        "w2qh": bf(np.ascontiguousarray(d["w2t"][:, :256])),
        "w2ql": bf(np.ascontiguousarray(d["w2t"][:, :256])
                   - np.asarray(bf(np.ascontiguousarray(d["w2t"][:, :256])),
                                np.float32)),
        "bi2": d["bi2"].astype(np.float32),
        "fw3": d["fw3"].astype(ml_dtypes.float8_e4m3),
        "fw5": d["fw5"].astype(ml_dtypes.float8_e4m3),
        "bc3": d["bc3"].astype(np.float32), "bc5": d["bc5"].astype(np.float32),
        "idt": bf(d["idt"]),
        "mp0": bf(d["mp0"]), "mp1": bf(d["mp1"]),
        "er": bf(d["er"]), "mdex": bf(d["mdex"]),
        "pw8": d["pw8"].astype(ml_dtypes.float8_e4m3),
        "pwid": bf(d["pwid"]),
        "pb": d["pb"].astype(np.float32),
        "svec": d["svec"].astype(np.float32),
        "w18": d["w18"].astype(ml_dtypes.float8_e4m3),
        "w2kv": bf(np.ascontiguousarray(d["w2t"][:, 256:])),
    }
    x = np.asarray(inputs["x"], np.float32).reshape(B, CIN, N)
    return base, x


def kernel(**inputs):
    from concourse.bass_utils import run_bass_kernel_spmd

    base, x = _feeds(inputs)
    in_maps = []
    for c in range(B):
        m = dict(base)
        m["xf"] = np.ascontiguousarray(x[c])
        in_maps.append(m)
    nc = _get_nc()
    res = run_bass_kernel_spmd(nc, in_maps, list(range(B))).results
    out = np.stack([np.asarray(r["out"]).reshape(CIN, HH, WW) for r in res])
    return out.astype(np.float32)



# revision 64
# speedup vs baseline: 1.0656x; 1.0039x over previous
"""LiteMLA (EfficientViT multi-scale linear attention) Trainium2 Bass kernel.

Sharding: data-parallel over batch B=8 across 8 NeuronCores (1 image/core).
Per-core pipeline:
  1. Streamed per-nt x: pass1 (bf16, natural channel order) -> zero-padded
     SBUF image for conv taps; pass2 -> attention Q buffer + id-scale K/V
     stages. The q channels use Dekker-split bf16 matmuls (whi.xhi +
     whi.xlo + wlo.xhi, ~16-bit effective mantissa): the id-scale heads are
     ill-conditioned (att = (vk@relu q)/(den@relu q) is 0/0 at positions
     where all 8 q dims are negative), so relu(q)'s sign pattern must track
     the fp32 reference closely; plain bf16 or HW-float32r inputs flip
     signs and cost ~0.15 rel err. k/v stay bf16 (4096-term averages).
     s3 conv is emitted interleaved into this loop to fill PE idle.
  2. s3/s5: depthwise 3x3/5x5 + grouped 1x1 FUSED on host into per-tap
     block-diagonal [96,96] weights; all taps of a block fetched in ONE
     sync-queue DMA; PE matmuls accumulate taps in PSUM reading shifted
     slices of the padded image.
  3. relu-linear attention: per spatial tile, relu(k)/v transposed by the
     DMA engines (xbar dma_start_transpose, contiguous [128,128] dst tiles)
     and reduced into per-16-head vk outer products; per-tile PSUM partials
     fold into SBUF accumulators on DVE; denominator = row-sums of relu(k).
  4. apply: att_raw = vk @ relu(q) via block-diagonal apply weights on raw
     Q (no per-position pre-scale, keeping PE free of the normalize chain);
     the batched [96,TN] denominator reciprocal is broadcast-expanded per
     region on PE and applied at the PSUM drain (one DVE multiply); proj
     contracts 128 rows (2 groups per matmul). Each tile's attention stage
     is processed one iteration deferred so its DMA-transpose issues overlap
     the next tile's matmuls.

All SBUF/PSUM operand slices start at partition 0/32/64/96 (HW requirement).
"""

import sys
import numpy as np

sys.path.insert(0, "/opt/trn_rl_repo")

B, CIN, HH, WW = 8, 256, 64, 64
N = HH * WW            # 4096
HEADS = 32             # per scale
C3 = 768
NHEADS = 96
PADW = WW + 4          # 68
NT = 8                 # spatial tiles of 512 positions (8 image rows each)
TN = 512
HALF = 2               # nts processed per conv weight fetch
TAPS3 = [(dy, dx) for dy in (-1, 0, 1) for dx in (-1, 0, 1)]
TAPS5 = [(dy, dx) for dy in (-2, -1, 0, 1, 2) for dx in (-2, -1, 0, 1, 2)]
NBLK = 8               # conv channel blocks of 4 head-groups
BLK = 96
NREG = 6               # vk regions of 16 heads

_cache = {}

PS = 16.0              # fp8 pad image pre-scale
WS = 256.0             # fp8 fused conv weight pre-scale
SC = 1.0 / (PS * WS)   # conv psum drain scale (2^-12)
NU = {3: 5, 5: 13}     # DoubleRow tap-pair units per scale


def _head_of(g12, i):
    return 16 * (g12 // 2) + 8 * (g12 % 2) + i


def _host_weights(inp):
    f32 = np.float32
    W = np.asarray(inp["qkv_w"], f32)[:, :, 0, 0]            # [768, 256]
    qkv_b = np.asarray(inp["qkv_b"], f32)
    pw = {3: np.asarray(inp["pw3_w"], f32)[:, :, 0, 0],
          5: np.asarray(inp["pw5_w"], f32)[:, :, 0, 0]}
    pwb = {3: np.asarray(inp["pw3_b"], f32), 5: np.asarray(inp["pw5_b"], f32)}
    dw = {3: np.asarray(inp["dw3_w"], f32)[:, 0],
          5: np.asarray(inp["dw5_w"], f32)[:, 0]}
    dwb = {3: np.asarray(inp["dw3_b"], f32), 5: np.asarray(inp["dw5_b"], f32)}
    proj_w = np.asarray(inp["proj_w"], f32)[:, :, 0, 0]      # [256, 768]
    proj_b = np.asarray(inp["proj_b"], f32)

    d = {}
    # pass1 weights pre-scaled by PS: pad fp8 image = PS*qkv via a pure-copy
    # drain (no scale op needed)
    d["w1t"] = np.ascontiguousarray(W.T) * PS                # [256, 768]
    # per-head reciprocal scale: attention is scale-invariant in q/k per head
    # and scales linearly with v, so conv drains skip the 1/(PS*WS) descale
    # and the v-scale is folded into the rcb copy (heads 32..95 are conv)
    sv = np.ones((NHEADS, 1), f32)
    sv[32:] = 1.0 / (PS * WS)
    # conv att pre-scaled 512x so its fp8 att tiles (DoubleRow proj) stay in
    # e4m3 normal range; id att stays bf16 (dominates output magnitude)
    sv[32:] *= 512.0
    d["svec"] = sv
    # pass1 fp8 DoubleRow weights: [NBLK, 128, 2, 96], *32 (drain scale .5*PS/16)
    w18 = np.empty((NBLK, 128, 2, BLK), f32)
    for b in range(NBLK):
        for j in range(2):
            w18[b, :, j, :] = 32.0 * W[BLK * b:BLK * (b + 1), 128 * j:128 * (j + 1)].T
    d["w18"] = w18
    perm2 = np.empty(768, np.int64)
    for h in range(HEADS):
        for e in range(8):
            perm2[h * 8 + e] = h * 24 + e
            perm2[256 + h * 8 + e] = h * 24 + 8 + e
            perm2[512 + h * 8 + e] = h * 24 + 16 + e
    d["w2t"] = np.ascontiguousarray(W[perm2].T)
    d["bi2"] = qkv_b[perm2].reshape(768, 1)

    # fused conv weights: per tap, 8 blocks of 4 groups, [96in, 96out q|k|v];
    # stored per-block contiguous over taps: [NBLK, 96in, taps*96out]
    for s, taps in ((3, TAPS3), (5, TAPS5)):
        fw = np.zeros((len(taps), NBLK, BLK, BLK), f32)
        fb = np.zeros((NBLK, BLK), f32)
        for b in range(NBLK):
            for gl in range(4):
                g = 4 * b + gl
                M24 = pw[s][g * 24:(g + 1) * 24]             # [24 out(oo), 24 in]
                bias24 = pwb[s][g * 24:(g + 1) * 24] + M24 @ dwb[s][g * 24:(g + 1) * 24]
                for oo in range(24):
                    m = (oo // 8) * 32 + gl * 8 + (oo % 8)   # [q32|k32|v32]
                    fb[b, m] = bias24[oo]
                dvec = dw[s][g * 24:(g + 1) * 24]            # [24 in, kh, kw]
                for ti, (dy, dx) in enumerate(taps):
                    wt = M24 * dvec[:, dy + s // 2, dx + s // 2][None, :]
                    for oo in range(24):
                        m = (oo // 8) * 32 + gl * 8 + (oo % 8)
                        fw[ti, b, gl * 24:(gl + 1) * 24, m] = wt[oo]
        # fp8 DoubleRow pair-stacked: [NBLK, BLK_in, U, 2, BLK_out]; unit u
        # holds taps (2u, 2u+1); odd tap count -> last unit slot 1 zeroed
        U = NU[s]
        fw8 = np.zeros((NBLK, BLK, U, 2, BLK), f32)
        for u in range(U):
            for j in range(2):
                ti = 2 * u + j
                if ti < len(taps):
                    fw8[:, :, u, j, :] = fw[ti] * WS
        d[f"fw{s}"] = fw8
        # conv drain bias in raw-psum units (outputs stay scaled by PS*WS)
        d[f"bc{s}"] = fb.reshape(NBLK, BLK, 1) * (PS * WS)

    d["idt"] = np.eye(128, dtype=f32)

    # masks for vk -> apply-weight assembly (dd-major cols, no den col)
    for half in range(2):
        mp = np.zeros((128, 64), f32)   # [(hp,e), (dd,h)]
        for p in range(128):
            hp = p // 8
            for h in range(8):
                if hp == h + 8 * half:
                    for dd in range(8):
                        mp[p, 8 * dd + h] = 1.0
        d[f"mp{half}"] = mp

    # expand matrices: er maps rc rows (96 heads) onto ap2's row layout
    # (col p: half=p//64, dd, i -> head 16r+8*(p//64)+p%8); mdex places den
    # values for the Q-region row layout (row p -> head 16r + p//8)
    er = np.zeros((NREG, 96, 128), f32)
    mdex = np.zeros((NREG, 128, 96), f32)
    for r in range(NREG):
        for p in range(128):
            er[r, 16 * r + 8 * (p // 64) + p % 8, p] = 1.0
            mdex[r, p, 16 * r + p // 8] = 1.0
    d["er"] = er
    d["mdex"] = mdex

    # proj lhsT [6, 128, 256]: rows 0:64 group 2r, 64:128 group 2r+1; row
    # (64*half + 8*dd + i) = proj col of head _head_of(2r+half, i), dim dd
    PW2 = np.zeros((NREG, 128, 256), f32)
    for r in range(NREG):
        for half in range(2):
            for i in range(8):
                Hh = _head_of(2 * r + half, i)
                for dd in range(8):
                    PW2[r, 64 * half + 8 * dd + i] = proj_w[:, 8 * Hh + dd]
    # proj: id regions 0,1 in bf16 at psum scale WP; conv region pairs
    # (2,3),(4,5) as fp8 DoubleRow k-tiles at WP/512 (att carries the 512)
    WP = 2048.0
    d["pwid"] = WP * PW2[0:2]
    pw8 = np.empty((2, 128, 2, 256), f32)
    for p in range(2):
        for j in range(2):
            pw8[p, :, j, :] = (WP / 512.0) * PW2[2 + 2 * p + j]
    d["pw8"] = pw8
    d["pb"] = proj_b.reshape(256, 1)
    return d


def _build():
    import concourse.bass as bass
    import concourse.bacc as bacc_mod
    import concourse.mybir as mybir
    from concourse.tile import TileContext

    dt = mybir.dt
    f32, bf16, f8 = dt.float32, dt.bfloat16, dt.float8e4
    AF = mybir.ActivationFunctionType
    ALU = mybir.AluOpType
    AX = mybir.AxisListType
    PM = mybir.MatmulPerfMode

    nc = bacc_mod.Bacc()
    x_in = nc.dram_tensor("xf", [CIN, N], f32, kind="ExternalInput")
    dW2QH = nc.dram_tensor("w2qh", [CIN, 256], bf16, kind="ExternalInput")
    dW2QL = nc.dram_tensor("w2ql", [CIN, 256], bf16, kind="ExternalInput")
    dBI2 = nc.dram_tensor("bi2", [C3, 1], f32, kind="ExternalInput")
    dFW3 = nc.dram_tensor("fw3", [NBLK, BLK, NU[3], 2, BLK], f8, kind="ExternalInput")
    dFW5 = nc.dram_tensor("fw5", [NBLK, BLK, NU[5], 2, BLK], f8, kind="ExternalInput")
    dBC3 = nc.dram_tensor("bc3", [NBLK, BLK, 1], f32, kind="ExternalInput")
    dBC5 = nc.dram_tensor("bc5", [NBLK, BLK, 1], f32, kind="ExternalInput")
    dIDT = nc.dram_tensor("idt", [128, 128], bf16, kind="ExternalInput")
    dMP = [nc.dram_tensor(f"mp{h}", [128, 64], bf16, kind="ExternalInput") for h in range(2)]
    dER = nc.dram_tensor("er", [NREG, 96, 128], bf16, kind="ExternalInput")
    dMDEX = nc.dram_tensor("mdex", [NREG, 128, 96], bf16, kind="ExternalInput")
    dPW8 = nc.dram_tensor("pw8", [2, 128, 2, 256], f8, kind="ExternalInput")
    dPWI = nc.dram_tensor("pwid", [2, 128, 256], bf16, kind="ExternalInput")
    dPB = nc.dram_tensor("pb", [256, 1], f32, kind="ExternalInput")
    dSV = nc.dram_tensor("svec", [NHEADS, 1], f32, kind="ExternalInput")
    dW18 = nc.dram_tensor("w18", [NBLK, 128, 2, BLK], f8, kind="ExternalInput")
    dW2kv = nc.dram_tensor("w2kv", [CIN, 512], bf16, kind="ExternalInput")
    d_out = nc.dram_tensor("out", [CIN, N], f32, kind="ExternalOutput")

    with TileContext(nc) as tc:
        with (
            tc.tile_pool(name="consts", bufs=1) as cpool,
            tc.tile_pool(name="persist", bufs=1) as qpool,
            tc.tile_pool(name="wstream", bufs=2) as wpool,
            tc.tile_pool(name="stage", bufs=2) as spool,
        ):
            # ---- constants (off the sync queue so x DMAs start immediately) --
            w2qh = [cpool.tile([128, 256], bf16, name=f"w2qh_{k}") for k in range(2)]
            w2ql = [cpool.tile([128, 256], bf16, name=f"w2ql_{k}") for k in range(2)]
            for k in range(2):
                nc.scalar.dma_start(out=w2qh[k][:], in_=dW2QH[128 * k:128 * (k + 1), :])
                nc.scalar.dma_start(out=w2ql[k][:], in_=dW2QL[128 * k:128 * (k + 1), :])
            bi2 = [cpool.tile([128, 1], f32, name=f"bi2_{j}") for j in range(6)]
            for j in range(6):
                nc.scalar.dma_start(out=bi2[j][:], in_=dBI2[128 * j:128 * (j + 1), :])
            # pass-1/2 weights FIRST: they gate the very first PE work;
            # the bulky conv weights follow (not needed until conv(0) ~15us in)
            w18t = cpool.tile([128, NBLK, 2, BLK], f8, name="w18t")
            for b in range(NBLK):
                nc.scalar.dma_start(out=w18t[:, b], in_=dW18[b])
            w2kv = [cpool.tile([128, 512], bf16, name=f"w2kv_{k}") for k in range(2)]
            for k in range(2):
                nc.scalar.dma_start(out=w2kv[k][:], in_=dW2kv[128 * k:128 * (k + 1), :])
            bc = {}
            for s, db in ((3, dBC3), (5, dBC5)):
                bc[s] = [cpool.tile([BLK, 1], f32, name=f"bc{s}_{b}") for b in range(NBLK)]
                for b in range(NBLK):
                    nc.gpsimd.dma_start(out=bc[s][b][:], in_=db[b])
            # resident fp8 DoubleRow conv weights: [96, U, 2, 96] per block
            cw = {}
            for s, dfw in ((3, dFW3), (5, dFW5)):
                cw[s] = [cpool.tile([BLK, NU[s], 2, BLK], f8, name=f"cw{s}_{b}")
                         for b in range(NBLK)]
                for b in range(NBLK):
                    eng = nc.scalar if b % 2 == 0 else nc.gpsimd
                    eng.dma_start(out=cw[s][b][:], in_=dfw[b])
            idt = cpool.tile([128, 128], bf16, name="idt")
            nc.scalar.dma_start(out=idt[:], in_=dIDT[:, :])
            mp = [cpool.tile([128, 64], bf16, name=f"mp_{h}") for h in range(2)]
            for h in range(2):
                nc.gpsimd.dma_start(out=mp[h][:], in_=dMP[h][:, :])
            ert = [cpool.tile([96, 128], bf16, name=f"er_{r}") for r in range(NREG)]
            mdex = [cpool.tile([128, 96], bf16, name=f"mdex_{r}") for r in range(NREG)]
            pw8t = [cpool.tile([128, 2, 256], f8, name=f"pw8_{p}") for p in range(2)]
            pwid = [cpool.tile([128, 256], bf16, name=f"pwid_{r}") for r in range(2)]
            for p in range(2):
                nc.gpsimd.dma_start(out=pw8t[p][:], in_=dPW8[p])
                nc.gpsimd.dma_start(out=pwid[p][:], in_=dPWI[p])
            for r in range(NREG):
                nc.gpsimd.dma_start(out=ert[r][:], in_=dER[r])
                nc.gpsimd.dma_start(out=mdex[r][:], in_=dMDEX[r])
            pbt = [cpool.tile([128, 1], f32, name=f"pbt_{m}") for m in range(2)]
            for m in range(2):
                nc.gpsimd.dma_start(out=pbt[m][:], in_=dPB[128 * m:128 * (m + 1), :])
            svt = cpool.tile([NHEADS, 1], f32, name="svt")
            nc.gpsimd.dma_start(out=svt[:], in_=dSV[:, :])
            epsw = cpool.tile([1, 96], bf16, name="epsw")
            nc.gpsimd.memset(epsw[:], 1e-15)
            ones1 = cpool.tile([1, TN], bf16, name="ones1")
            nc.gpsimd.memset(ones1[:], 1.0)

            # ---- persistent activations ----
            pad = [qpool.tile([BLK, PADW, PADW], f8, name=f"pad_{b}") for b in range(NBLK)]
            for b in range(NBLK):
                # zero only the 2-wide borders; interior is fully written by
                # pass 1 (keeps these memsets off pass 1's dependency chain)
                nc.gpsimd.memset(pad[b][:, 0:2, :], 0.0)
                nc.gpsimd.memset(pad[b][:, PADW - 2:PADW, :], 0.0)
                nc.gpsimd.memset(pad[b][:, 2:PADW - 2, 0:2], 0.0)
                nc.gpsimd.memset(pad[b][:, 2:PADW - 2, PADW - 2:PADW], 0.0)
            Q = [qpool.tile([128, N], bf16, name=f"Q_{r}") for r in range(NREG)]
            kpart = [qpool.tile([128, NT], f32, name=f"kpart_{r}") for r in range(NREG)]
            vks_sb = [qpool.tile([128, 128], bf16, name=f"vks_{r}") for r in range(NREG)]

            vks_acc = [qpool.tile([128, 128], f32, name=f"vka_{r}")
                       for r in range(NREG)]

            # PSUM pool A: phases 1 + conv (reclaimed before apply phase)
            _pA = tc.tile_pool(name="psumA", bufs=2, space="PSUM")
            ppool = _pA.__enter__()

            # ============ shared per-tile attention stage ====================
            def process_stage(s_idx, nt, ks, vs):
                # first two id-stages: PE transposes (PE is idle pre-conv and
                # this keeps DMA-transfer latency off the early critical path)
                pe_tr = False
                """ks/vs: 2 bf16 [128,512] stage tiles (relu'd k / raw v)."""
                for t in range(2):
                    r = 2 * s_idx + t
                    nc.vector.reduce_sum(out=kpart[r][:, nt:nt + 1], in_=ks[t][:], axis=AX.X)
                for jj in range(4):
                    for t in range(2):
                        kT = spool.tile([128, 128], bf16, name="kT", tag="kT", bufs=12)
                        vT = spool.tile([128, 128], bf16, name="vT", tag="vT", bufs=12)
                        if pe_tr:
                            tp = ppool.tile([128, 128], bf16, name="tp", tag="mm",
                                            bufs=2)
                            nc.tensor.transpose(
                                tp[:], ks[t][:, 128 * jj:128 * (jj + 1)], idt[:])
                            nc.scalar.copy(out=kT[:], in_=tp[:])
                            tp2 = ppool.tile([128, 128], bf16, name="tp2", tag="mm",
                                             bufs=2)
                            nc.tensor.transpose(
                                tp2[:], vs[t][:, 128 * jj:128 * (jj + 1)], idt[:])
                            nc.vector.tensor_copy(out=vT[:], in_=tp2[:])
                        else:
                            nc.sync.dma_start_transpose(
                                out=kT[:], in_=ks[t][:, 128 * jj:128 * (jj + 1)])
                            nc.sync.dma_start_transpose(
                                out=vT[:], in_=vs[t][:, 128 * jj:128 * (jj + 1)])
                        nc.tensor.matmul(vkps[t][:], kT[:], vT[:],
                                         start=(jj == 0), stop=(jj == 3))
                for t in range(2):
                    r = 2 * s_idx + t
                    if nt == 0:
                        nc.vector.tensor_copy(out=vks_acc[r][:], in_=vkps[t][:])
                    else:
                        nc.vector.tensor_tensor(out=vks_acc[r][:], in0=vks_acc[r][:],
                                                in1=vkps[t][:], op=ALU.add)
                    if nt == NT - 1:
                        nc.gpsimd.tensor_copy(out=vks_sb[r][:], in_=vks_acc[r][:])

            # ===== pass 1+2 per nt: stream x, build pad image + id stage =====

            def emit_pass12(nt):
                xt = [spool.tile([128, TN], f32, name="xt", tag=f"xt{k}", bufs=2)
                      for k in range(2)]
                xb = [spool.tile([128, TN], bf16, name="xb", tag=f"xb{k}", bufs=2)
                      for k in range(2)]
                xl = [spool.tile([128, TN], bf16, name="xl", tag=f"xl{k}", bufs=2)
                      for k in range(2)]
                xf8 = spool.tile([128, 2, TN], f8, name="xf8", tag="xf8", bufs=2)
                for k in range(2):
                    nc.sync.dma_start(out=xt[k][:],
                                      in_=x_in[128 * k:128 * (k + 1), TN * nt:TN * (nt + 1)])
                    nc.gpsimd.tensor_copy(out=xb[k][:], in_=xt[k][:])
                    nc.gpsimd.tensor_tensor(out=xl[k][:], in0=xt[k][:], in1=xb[k][:],
                                            op=ALU.subtract)
                    nc.gpsimd.tensor_copy(out=xf8[:, k, :], in_=xt[k][:])
                # pass 1 (fp8 DoubleRow): natural order -> padded fp8 image;
                # psum is 32*qkv, pad stores PS*qkv -> drain scale 0.5
                for b in range(NBLK):
                    ps = ppool.tile([BLK, 8, WW], f32, name="ps1", tag="mm", bufs=2)
                    nc.tensor.matmul(ps[:].rearrange("p a c -> p (a c)"),
                                     w18t[:, b], xf8[:],
                                     start=True, stop=True,
                                     perf_mode=PM.DoubleRow)
                    dst = pad[b][:, 2 + 8 * nt:10 + 8 * nt, 2:2 + WW]
                    if b % 2 == 0:
                        nc.scalar.activation(out=dst, in_=ps[:], func=AF.Copy,
                                             bias=0.0, scale=PS / 32.0)
                    else:
                        nc.vector.tensor_scalar(out=dst, in0=ps[:],
                                                scalar1=PS / 32.0, scalar2=None,
                                                op0=ALU.mult)
                # pass 2: separated order (fp32r) -> Q + id-scale k/v stages
                ks, vs = [None, None], [None, None]
                for j in range(6):
                    ps = ppool.tile([128, TN], f32, name="ps2", tag="mm", bufs=2)
                    if j < 2:
                        for k in range(2):
                            sl_w = slice(128 * j, 128 * (j + 1))
                            nc.tensor.matmul(ps[:], w2qh[k][:, sl_w], xb[k][:],
                                             start=(k == 0), stop=False)
                            nc.tensor.matmul(ps[:], w2qh[k][:, sl_w], xl[k][:],
                                             start=False, stop=False)
                            nc.tensor.matmul(ps[:], w2ql[k][:, sl_w], xb[k][:],
                                             start=False, stop=(k == 1))
                    else:
                        for k in range(2):
                            nc.tensor.matmul(
                                ps[:], w2kv[k][:, 128 * (j - 2):128 * (j - 1)],
                                xb[k][:], start=(k == 0), stop=(k == 1))
                    if j < 2:
                        nc.scalar.activation(out=Q[j][:, TN * nt:TN * (nt + 1)], in_=ps[:],
                                             func=AF.Relu, bias=bi2[j][:], scale=1.0)
                    elif j < 4:
                        t = j - 2
                        kst = spool.tile([128, TN], bf16, name="ks", tag=f"ks{t}", bufs=4)
                        nc.scalar.activation(out=kst[:], in_=ps[:], func=AF.Relu,
                                             bias=bi2[j][:], scale=1.0)
                        ks[t] = kst
                    else:
                        t = j - 4
                        vst = spool.tile([128, TN], bf16, name="vs", tag=f"vs{t}", bufs=4)
                        nc.vector.tensor_scalar(out=vst[:], in0=ps[:], scalar1=bi2[j][:],
                                                scalar2=None, op0=ALU.add)
                        vs[t] = vst
                return ks, vs

            # ================= fused conv scales (fp8 DoubleRow) =============
            # tap pair u = row-major taps (2u, 2u+1); pair delta in the padded
            # image is off(2u+1)-off(2u); odd tail pairs (tap, tap) with
            # zeroed slot-1 weights (stride 0).
            def pair_deltas(taps):
                ds = []
                for u in range((len(taps) + 1) // 2):
                    if 2 * u + 1 < len(taps):
                        dy0, dx0 = taps[2 * u]
                        dy1, dx1 = taps[2 * u + 1]
                        ds.append((dy1 - dy0) * PADW + (dx1 - dx0))
                    else:
                        ds.append(0)
                return ds

            DELTAS = {3: pair_deltas(TAPS3), 5: pair_deltas(TAPS5)}

            def emit_conv_nt(nt):
                """Both conv scales for one spatial tile, all 8 blocks, then
                their attention stages."""
                stg = {}
                for s_idx in (1, 2):
                    for t in range(2):
                        stg[("k", s_idx, t)] = spool.tile(
                            [128, TN], bf16, name="ks", tag=f"ks{t}", bufs=4)
                        stg[("v", s_idx, t)] = spool.tile(
                            [128, TN], bf16, name="vs", tag=f"vs{t}", bufs=4)
                for b in range(NBLK):
                    for s, s_idx in ((3, 1), (5, 2)):
                        taps, U = TAPS3 if s == 3 else TAPS5, NU[s]
                        cp = ppool.tile([BLK, 8, WW], f32, name="cp", tag="conv",
                                        bufs=4)
                        # per image row: CoreSim's DoubleRow path needs the
                        # rhs to view as exactly [p, 2, N]
                        for u in range(U):
                            dy0, dx0 = taps[2 * u]
                            for r in range(8):
                                rhs = pad[b][:, 2 + 8 * nt + dy0 + r,
                                             2 + dx0:2 + dx0 + WW].copy()
                                rhs.ap.insert(1, [DELTAS[s][u], 2])
                                nc.tensor.matmul(cp[:, r], cw[s][b][:, u], rhs,
                                                 start=(u == 0 and r == 0),
                                                 stop=(u == U - 1 and r == 7),
                                                 perf_mode=PM.DoubleRow)
                        qt, qr = (256 * s_idx + 32 * b) // 128, (32 * b) % 128
                        t2, r2 = b // 4, (32 * b) % 128
                        # drains stay in psum scale (PS*WS); q/k scales cancel
                        # in the attention ratio, v scale folds into rcb
                        nc.scalar.activation(
                            out=Q[qt][qr:qr + 32, TN * nt:TN * (nt + 1)],
                            in_=cp[0:32].rearrange("p a c -> p (a c)"),
                            func=AF.Relu, bias=bc[s][b][0:32, :], scale=1.0)
                        if b >= 7:
                            nc.scalar.activation(
                                out=stg[("k", s_idx, t2)][r2:r2 + 32, :],
                                in_=cp[32:64].rearrange("p a c -> p (a c)"),
                                func=AF.Relu, bias=bc[s][b][32:64, :], scale=1.0)
                        else:
                            nc.vector.tensor_scalar(
                                out=stg[("k", s_idx, t2)][r2:r2 + 32, :],
                                in0=cp[32:64].rearrange("p a c -> p (a c)"),
                                scalar1=bc[s][b][32:64, :], scalar2=0.0,
                                op0=ALU.add, op1=ALU.max)
                        if b % 2 == 0:
                            nc.scalar.activation(
                                out=stg[("v", s_idx, t2)][r2:r2 + 32, :],
                                in_=cp[64:96].rearrange("p a c -> p (a c)"),
                                func=AF.Identity, bias=bc[s][b][64:96, :],
                                scale=1.0)
                        else:
                            nc.vector.tensor_scalar(
                                out=stg[("v", s_idx, t2)][r2:r2 + 32, :],
                                in0=cp[64:96].rearrange("p a c -> p (a c)"),
                                scalar1=bc[s][b][64:96, :], scalar2=None,
                                op0=ALU.add)
                for s_idx in (1, 2):
                    process_stage(s_idx, nt,
                                  [stg[("k", s_idx, t)] for t in range(2)],
                                  [stg[("v", s_idx, t)] for t in range(2)])
                    if nt == NT - 1 and s_idx == 1:
                        emit_asm(2)
                        emit_asm(3)

            # Stream: pass12(nt) feeds pad rows; conv for nt-1 is ready once
            # pass 1 has written rows through nt (s5 needs dy<=+2).
            prev_stage = None
            for nt in range(NT):
                ksvs = emit_pass12(nt)
                if prev_stage is not None:
                    process_stage(0, nt - 1, *prev_stage)
                prev_stage = ksvs
                if nt >= 1:
                    emit_conv_nt(nt - 1)
            process_stage(0, NT - 1, *prev_stage)
            emit_conv_nt(NT - 1)

            # ====== assemble apply weights + denominator lhsT from vk ========
            appw = []
            denw = []
            for r in range(NREG):
                kf = qpool.tile([128, 1], f32, name=f"kfin_{r}")
                nc.vector.reduce_sum(out=kf[:], in_=kpart[r][:], axis=AX.X)
                dwt = qpool.tile([128, 96], bf16, name=f"denw_{r}")
                nc.gpsimd.tensor_scalar(out=dwt[:], in0=mdex[r][:],
                                        scalar1=kf[:], scalar2=None, op0=ALU.mult)
                denw.append(dwt)
                vks = vks_sb[r]
                aw = qpool.tile([128, 128], bf16, name=f"appw_{r}")
                for half in range(2):
                    nc.gpsimd.tensor_tensor(
                        out=aw[:, 64 * half:64 * (half + 1)].rearrange(
                            "p (d h) -> p d h", h=8),
                        in0=vks[:, 64 * half:64 * (half + 1)].rearrange(
                            "p (h d) -> p d h", d=8),
                        in1=mp[half][:].rearrange("p (d h) -> p d h", h=8),
                        op=ALU.mult)
                appw.append(aw)

            # ====== apply (pre-normalized q) + proj ==========================
            _pA.__exit__(None, None, None)
            _pd.__exit__(None, None, None)
            _pB = tc.tile_pool(name="psumB", bufs=2, space="PSUM")
            ppb = _pB.__enter__()

            def emit_ddp(nt):
                """denominator dd[h, n] = den_h . q~_h(n) + eps, all 96 heads"""
                sl = slice(TN * nt, TN * (nt + 1))
                ddp = ppb.tile([96, TN], f32, name="ddp", tag="dd", bufs=2)
                nc.tensor.matmul(ddp[:], epsw[:], ones1[:], start=True, stop=False)
                for r in range(NREG):
                    nc.tensor.matmul(ddp[:], denw[r][:], Q[r][:, sl],
                                     start=False, stop=(r == NREG - 1))
                return ddp

            ddp_cur = emit_ddp(0)
            for nt in range(NT):
                sl = slice(TN * nt, TN * (nt + 1))
                rc = spool.tile([96, TN], f32, name="rc", tag="rc", bufs=1)
                nc.vector.reciprocal_approx_fast(out=rc[:], in_=ddp_cur[:])
                rcb = spool.tile([96, TN], bf16, name="rcb", tag="rcb", bufs=1)
                # fold the per-head v-scale (1 id / SC conv) into the copy
                nc.scalar.activation(out=rcb[:], in_=rc[:], func=AF.Copy,
                                     bias=0.0, scale=svt[:])
                atid = []
                at2 = [None, None]
                for r in range(NREG):
                    rcx = ppb.tile([128, TN], f32, name="rcx", tag="rcx", bufs=2)
                    nc.tensor.matmul(rcx[:], ert[r][:], rcb[:], start=True, stop=True)
                    rxb = spool.tile([128, TN], bf16, name="rxb", tag="rxb", bufs=6)
                    nc.scalar.copy(out=rxb[:], in_=rcx[:])
                    ap2 = ppb.tile([128, TN], f32, name="ap2", tag="ap2", bufs=2)
                    nc.tensor.matmul(ap2[:], appw[r][:], Q[r][:, sl],
                                     start=True, stop=True)
                    if r < 2:
                        at = spool.tile([128, TN], bf16, name="at", tag="at",
                                        bufs=3)
                        nc.vector.tensor_tensor(out=at[:], in0=ap2[:],
                                                in1=rxb[:], op=ALU.mult)
                        atid.append(at)
                    else:
                        p = (r - 2) // 2
                        if r % 2 == 0:
                            at2[p] = spool.tile([128, 2, TN], f8, name="at2",
                                                tag="at2", bufs=3)
                        nc.vector.tensor_tensor(out=at2[p][:, r % 2], in0=ap2[:],
                                                in1=rxb[:], op=ALU.mult)
                # hoist next tile's denominator matmuls here so the at-mults
                # (DVE) finish while PE runs them; pj then starts unstalled
                ddp_next = emit_ddp(nt + 1) if nt + 1 < NT else None
                for m in range(2):
                    pj = ppb.tile([128, TN], f32, name="pj", tag="pj", bufs=2)
                    for r in range(2):
                        nc.tensor.matmul(pj[:], pwid[r][:, 128 * m:128 * (m + 1)],
                                         atid[r][:], start=(r == 0), stop=False)
                    for p in range(2):
                        nc.tensor.matmul(pj[:], pw8t[p][:, :, 128 * m:128 * (m + 1)],
                                         at2[p][:], start=False, stop=(p == 1),
                                         perf_mode=PM.DoubleRow)
                    ob = spool.tile([128, TN], f32, name="ob", tag="ob", bufs=2)
                    nc.scalar.activation(out=ob[:], in_=pj[:], func=AF.Identity,
                                         bias=pbt[m][:], scale=1.0 / 2048.0)
                    nc.sync.dma_start(
                        out=d_out[128 * m:128 * (m + 1), TN * nt:TN * (nt + 1)], in_=ob[:])
                ddp_cur = ddp_next
            _pB.__exit__(None, None, None)
    return nc


def _get_nc():
    if "nc" not in _cache:
        nc = _build()
        nc.compile()
        _cache["nc"] = nc
    return _cache["nc"]


def _feeds(inputs):
    import ml_dtypes

    def bf(a):
        return np.asarray(a, np.float32).astype(ml_dtypes.bfloat16)

    d = _host_weights(inputs)
    base = {
        "w2qh": bf(np.ascontiguousarray(d["w2t"][:, :256])),
        "w2ql": bf(np.ascontiguousarray(d["w2t"][:, :256])
                   - np.asarray(bf(np.ascontiguousarray(d["w2t"][:, :256])),
                                np.float32)),
        "bi2": d["bi2"].astype(np.float32),
        "fw3": d["fw3"].astype(ml_dtypes.float8_e4m3),
        "fw5": d["fw5"].astype(ml_dtypes.float8_e4m3),
        "bc3": d["bc3"].astype(np.float32), "bc5": d["bc5"].astype(np.float32),
        "idt": bf(d["idt"]),
        "mp0": bf(d["mp0"]), "mp1": bf(d["mp1"]),
        "er": bf(d["er"]), "mdex": bf(d["mdex"]),
        "pw8": d["pw8"].astype(ml_dtypes.float8_e4m3),
        "pwid": bf(d["pwid"]),
        "pb": d["pb"].astype(np.float32),
        "svec": d["svec"].astype(np.float32),
        "w18": d["w18"].astype(ml_dtypes.float8_e4m3),
        "w2kv": bf(np.ascontiguousarray(d["w2t"][:, 256:])),
    }
    x = np.asarray(inputs["x"], np.float32).reshape(B, CIN, N)
    return base, x


def kernel(**inputs):
    from concourse.bass_utils import run_bass_kernel_spmd

    base, x = _feeds(inputs)
    in_maps = []
    for c in range(B):
        m = dict(base)
        m["xf"] = np.ascontiguousarray(x[c])
        in_maps.append(m)
    nc = _get_nc()
    res = run_bass_kernel_spmd(nc, in_maps, list(range(B))).results
    out = np.stack([np.asarray(r["out"]).reshape(CIN, HH, WW) for r in res])
    return out.astype(np.float32)



# revision 65
# speedup vs baseline: 1.1113x; 1.0428x over previous
"""LiteMLA (EfficientViT multi-scale linear attention) Trainium2 Bass kernel.

Sharding: data-parallel over batch B=8 across 8 NeuronCores (1 image/core).
Per-core pipeline:
  1. Streamed per-nt x: pass1 (bf16, natural channel order) -> zero-padded
     SBUF image for conv taps; pass2 -> attention Q buffer + id-scale K/V
     stages. The q channels use Dekker-split bf16 matmuls (whi.xhi +
     whi.xlo + wlo.xhi, ~16-bit effective mantissa): the id-scale heads are
     ill-conditioned (att = (vk@relu q)/(den@relu q) is 0/0 at positions
     where all 8 q dims are negative), so relu(q)'s sign pattern must track
     the fp32 reference closely; plain bf16 or HW-float32r inputs flip
     signs and cost ~0.15 rel err. k/v stay bf16 (4096-term averages).
     s3 conv is emitted interleaved into this loop to fill PE idle.
  2. s3/s5: depthwise 3x3/5x5 + grouped 1x1 FUSED on host into per-tap
     block-diagonal [96,96] weights; all taps of a block fetched in ONE
     sync-queue DMA; PE matmuls accumulate taps in PSUM reading shifted
     slices of the padded image.
  3. relu-linear attention: per spatial tile, relu(k)/v transposed by the
     DMA engines (xbar dma_start_transpose, contiguous [128,128] dst tiles)
     and reduced into per-16-head vk outer products; per-tile PSUM partials
     fold into SBUF accumulators on DVE; denominator = row-sums of relu(k).
  4. apply: att_raw = vk @ relu(q) via block-diagonal apply weights on raw
     Q (no per-position pre-scale, keeping PE free of the normalize chain);
     the batched [96,TN] denominator reciprocal is broadcast-expanded per
     region on PE and applied at the PSUM drain (one DVE multiply); proj
     contracts 128 rows (2 groups per matmul). Each tile's attention stage
     is processed one iteration deferred so its DMA-transpose issues overlap
     the next tile's matmuls.

All SBUF/PSUM operand slices start at partition 0/32/64/96 (HW requirement).
"""

import sys
import numpy as np

sys.path.insert(0, "/opt/trn_rl_repo")

B, CIN, HH, WW = 8, 256, 64, 64
N = HH * WW            # 4096
HEADS = 32             # per scale
C3 = 768
NHEADS = 96
PADW = WW + 4          # 68
NT = 8                 # spatial tiles of 512 positions (8 image rows each)
TN = 512
HALF = 2               # nts processed per conv weight fetch
TAPS3 = [(dy, dx) for dy in (-1, 0, 1) for dx in (-1, 0, 1)]
TAPS5 = [(dy, dx) for dy in (-2, -1, 0, 1, 2) for dx in (-2, -1, 0, 1, 2)]
NBLK = 8               # conv channel blocks of 4 head-groups
BLK = 96
NREG = 6               # vk regions of 16 heads

_cache = {}

PS = 16.0              # fp8 pad image pre-scale
WS = 256.0             # fp8 fused conv weight pre-scale
SC = 1.0 / (PS * WS)   # conv psum drain scale (2^-12)
NU = {3: 5, 5: 13}     # DoubleRow tap-pair units per scale


def _head_of(g12, i):
    return 16 * (g12 // 2) + 8 * (g12 % 2) + i


def _host_weights(inp):
    f32 = np.float32
    W = np.asarray(inp["qkv_w"], f32)[:, :, 0, 0]            # [768, 256]
    qkv_b = np.asarray(inp["qkv_b"], f32)
    pw = {3: np.asarray(inp["pw3_w"], f32)[:, :, 0, 0],
          5: np.asarray(inp["pw5_w"], f32)[:, :, 0, 0]}
    pwb = {3: np.asarray(inp["pw3_b"], f32), 5: np.asarray(inp["pw5_b"], f32)}
    dw = {3: np.asarray(inp["dw3_w"], f32)[:, 0],
          5: np.asarray(inp["dw5_w"], f32)[:, 0]}
    dwb = {3: np.asarray(inp["dw3_b"], f32), 5: np.asarray(inp["dw5_b"], f32)}
    proj_w = np.asarray(inp["proj_w"], f32)[:, :, 0, 0]      # [256, 768]
    proj_b = np.asarray(inp["proj_b"], f32)

    d = {}
    # pass1 weights pre-scaled by PS: pad fp8 image = PS*qkv via a pure-copy
    # drain (no scale op needed)
    d["w1t"] = np.ascontiguousarray(W.T) * PS                # [256, 768]
    # per-head reciprocal scale: attention is scale-invariant in q/k per head
    # and scales linearly with v, so conv drains skip the 1/(PS*WS) descale
    # and the v-scale is folded into the rcb copy (heads 32..95 are conv)
    sv = np.ones((NHEADS, 1), f32)
    sv[32:] = 1.0 / (PS * WS)
    # conv att pre-scaled 512x so its fp8 att tiles (DoubleRow proj) stay in
    # e4m3 normal range; id att stays bf16 (dominates output magnitude)
    sv[32:] *= 512.0
    d["svec"] = sv
    # pass1 fp8 DoubleRow weights: [NBLK, 128, 2, 96], *32 (drain scale .5*PS/16)
    w18 = np.empty((NBLK, 128, 2, BLK), f32)
    for b in range(NBLK):
        for j in range(2):
            w18[b, :, j, :] = 32.0 * W[BLK * b:BLK * (b + 1), 128 * j:128 * (j + 1)].T
    d["w18"] = w18
    perm2 = np.empty(768, np.int64)
    for h in range(HEADS):
        for e in range(8):
            perm2[h * 8 + e] = h * 24 + e
            perm2[256 + h * 8 + e] = h * 24 + 8 + e
            perm2[512 + h * 8 + e] = h * 24 + 16 + e
    d["w2t"] = np.ascontiguousarray(W[perm2].T)
    d["bi2"] = qkv_b[perm2].reshape(768, 1)

    # fused conv weights: per tap, 8 blocks of 4 groups, [96in, 96out q|k|v];
    # stored per-block contiguous over taps: [NBLK, 96in, taps*96out]
    for s, taps in ((3, TAPS3), (5, TAPS5)):
        fw = np.zeros((len(taps), NBLK, BLK, BLK), f32)
        fb = np.zeros((NBLK, BLK), f32)
        for b in range(NBLK):
            for gl in range(4):
                g = 4 * b + gl
                M24 = pw[s][g * 24:(g + 1) * 24]             # [24 out(oo), 24 in]
                bias24 = pwb[s][g * 24:(g + 1) * 24] + M24 @ dwb[s][g * 24:(g + 1) * 24]
                for oo in range(24):
                    m = (oo // 8) * 32 + gl * 8 + (oo % 8)   # [q32|k32|v32]
                    fb[b, m] = bias24[oo]
                dvec = dw[s][g * 24:(g + 1) * 24]            # [24 in, kh, kw]
                for ti, (dy, dx) in enumerate(taps):
                    wt = M24 * dvec[:, dy + s // 2, dx + s // 2][None, :]
                    for oo in range(24):
                        m = (oo // 8) * 32 + gl * 8 + (oo % 8)
                        fw[ti, b, gl * 24:(gl + 1) * 24, m] = wt[oo]
        # fp8 DoubleRow pair-stacked: [NBLK, BLK_in, U, 2, BLK_out]; unit u
        # holds taps (2u, 2u+1); odd tap count -> last unit slot 1 zeroed
        U = NU[s]
        fw8 = np.zeros((NBLK, BLK, U, 2, BLK), f32)
        for u in range(U):
            for j in range(2):
                ti = 2 * u + j
                if ti < len(taps):
                    fw8[:, :, u, j, :] = fw[ti] * WS
        d[f"fw{s}"] = fw8
        # conv drain bias in raw-psum units (outputs stay scaled by PS*WS)
        d[f"bc{s}"] = fb.reshape(NBLK, BLK, 1) * (PS * WS)

    d["idt"] = np.eye(128, dtype=f32)

    # masks for vk -> apply-weight assembly (dd-major cols, no den col)
    for half in range(2):
        mp = np.zeros((128, 64), f32)   # [(hp,e), (dd,h)]
        for p in range(128):
            hp = p // 8
            for h in range(8):
                if hp == h + 8 * half:
                    for dd in range(8):
                        mp[p, 8 * dd + h] = 1.0
        d[f"mp{half}"] = mp

    # expand matrices: er maps rc rows (96 heads) onto ap2's row layout
    # (col p: half=p//64, dd, i -> head 16r+8*(p//64)+p%8); mdex places den
    # values for the Q-region row layout (row p -> head 16r + p//8)
    er = np.zeros((NREG, 96, 128), f32)
    mdex = np.zeros((NREG, 128, 96), f32)
    for r in range(NREG):
        for p in range(128):
            er[r, 16 * r + 8 * (p // 64) + p % 8, p] = 1.0
            mdex[r, p, 16 * r + p // 8] = 1.0
    d["er"] = er
    d["mdex"] = mdex

    # proj lhsT [6, 128, 256]: rows 0:64 group 2r, 64:128 group 2r+1; row
    # (64*half + 8*dd + i) = proj col of head _head_of(2r+half, i), dim dd
    PW2 = np.zeros((NREG, 128, 256), f32)
    for r in range(NREG):
        for half in range(2):
            for i in range(8):
                Hh = _head_of(2 * r + half, i)
                for dd in range(8):
                    PW2[r, 64 * half + 8 * dd + i] = proj_w[:, 8 * Hh + dd]
    # proj: id regions 0,1 in bf16 at psum scale WP; conv region pairs
    # (2,3),(4,5) as fp8 DoubleRow k-tiles at WP/512 (att carries the 512)
    WP = 2048.0
    d["pwid"] = WP * PW2[0:2]
    pw8 = np.empty((2, 128, 2, 256), f32)
    for p in range(2):
        for j in range(2):
            pw8[p, :, j, :] = (WP / 512.0) * PW2[2 + 2 * p + j]
    d["pw8"] = pw8
    d["pb"] = proj_b.reshape(256, 1)
    return d


def _build():
    import concourse.bass as bass
    import concourse.bacc as bacc_mod
    import concourse.mybir as mybir
    from concourse.tile import TileContext

    dt = mybir.dt
    f32, bf16, f8 = dt.float32, dt.bfloat16, dt.float8e4
    AF = mybir.ActivationFunctionType
    ALU = mybir.AluOpType
    AX = mybir.AxisListType
    PM = mybir.MatmulPerfMode

    nc = bacc_mod.Bacc()
    x_in = nc.dram_tensor("xf", [CIN, N], f32, kind="ExternalInput")
    dW2QH = nc.dram_tensor("w2qh", [CIN, 256], bf16, kind="ExternalInput")
    dW2QL = nc.dram_tensor("w2ql", [CIN, 256], bf16, kind="ExternalInput")
    dBI2 = nc.dram_tensor("bi2", [C3, 1], f32, kind="ExternalInput")
    dFW3 = nc.dram_tensor("fw3", [NBLK, BLK, NU[3], 2, BLK], f8, kind="ExternalInput")
    dFW5 = nc.dram_tensor("fw5", [NBLK, BLK, NU[5], 2, BLK], f8, kind="ExternalInput")
    dBC3 = nc.dram_tensor("bc3", [NBLK, BLK, 1], f32, kind="ExternalInput")
    dBC5 = nc.dram_tensor("bc5", [NBLK, BLK, 1], f32, kind="ExternalInput")
    dIDT = nc.dram_tensor("idt", [128, 128], bf16, kind="ExternalInput")
    dMP = [nc.dram_tensor(f"mp{h}", [128, 64], bf16, kind="ExternalInput") for h in range(2)]
    dER = nc.dram_tensor("er", [NREG, 96, 128], bf16, kind="ExternalInput")
    dMDEX = nc.dram_tensor("mdex", [NREG, 128, 96], bf16, kind="ExternalInput")
    dPW8 = nc.dram_tensor("pw8", [2, 128, 2, 256], f8, kind="ExternalInput")
    dPWI = nc.dram_tensor("pwid", [2, 128, 256], bf16, kind="ExternalInput")
    dPB = nc.dram_tensor("pb", [256, 1], f32, kind="ExternalInput")
    dSV = nc.dram_tensor("svec", [NHEADS, 1], f32, kind="ExternalInput")
    dW18 = nc.dram_tensor("w18", [NBLK, 128, 2, BLK], f8, kind="ExternalInput")
    dW2kv = nc.dram_tensor("w2kv", [CIN, 512], bf16, kind="ExternalInput")
    d_out = nc.dram_tensor("out", [CIN, N], f32, kind="ExternalOutput")

    with TileContext(nc) as tc:
        with (
            tc.tile_pool(name="consts", bufs=1) as cpool,
            tc.tile_pool(name="persist", bufs=1) as qpool,
            tc.tile_pool(name="wstream", bufs=2) as wpool,
            tc.tile_pool(name="stage", bufs=2) as spool,
        ):
            # ---- constants (off the sync queue so x DMAs start immediately) --
            w2qh = [cpool.tile([128, 256], bf16, name=f"w2qh_{k}") for k in range(2)]
            w2ql = [cpool.tile([128, 256], bf16, name=f"w2ql_{k}") for k in range(2)]
            for k in range(2):
                nc.scalar.dma_start(out=w2qh[k][:], in_=dW2QH[128 * k:128 * (k + 1), :])
                nc.scalar.dma_start(out=w2ql[k][:], in_=dW2QL[128 * k:128 * (k + 1), :])
            bi2 = [cpool.tile([128, 1], f32, name=f"bi2_{j}") for j in range(6)]
            for j in range(6):
                nc.scalar.dma_start(out=bi2[j][:], in_=dBI2[128 * j:128 * (j + 1), :])
            # pass-1/2 weights FIRST: they gate the very first PE work;
            # the bulky conv weights follow (not needed until conv(0) ~15us in)
            w18t = cpool.tile([128, NBLK, 2, BLK], f8, name="w18t")
            for b in range(NBLK):
                nc.scalar.dma_start(out=w18t[:, b], in_=dW18[b])
            w2kv = [cpool.tile([128, 512], bf16, name=f"w2kv_{k}") for k in range(2)]
            for k in range(2):
                nc.scalar.dma_start(out=w2kv[k][:], in_=dW2kv[128 * k:128 * (k + 1), :])
            bc = {}
            for s, db in ((3, dBC3), (5, dBC5)):
                bc[s] = [cpool.tile([BLK, 1], f32, name=f"bc{s}_{b}") for b in range(NBLK)]
                for b in range(NBLK):
                    nc.gpsimd.dma_start(out=bc[s][b][:], in_=db[b])
            # resident fp8 DoubleRow conv weights: [96, U, 2, 96] per block
            cw = {}
            for s, dfw in ((3, dFW3), (5, dFW5)):
                cw[s] = [cpool.tile([BLK, NU[s], 2, BLK], f8, name=f"cw{s}_{b}")
                         for b in range(NBLK)]
                for b in range(NBLK):
                    eng = nc.scalar if b % 2 == 0 else nc.gpsimd
                    eng.dma_start(out=cw[s][b][:], in_=dfw[b])
            idt = cpool.tile([128, 128], bf16, name="idt")
            nc.scalar.dma_start(out=idt[:], in_=dIDT[:, :])
            mp = [cpool.tile([128, 64], bf16, name=f"mp_{h}") for h in range(2)]
            for h in range(2):
                nc.gpsimd.dma_start(out=mp[h][:], in_=dMP[h][:, :])
            ert = [cpool.tile([96, 128], bf16, name=f"er_{r}") for r in range(NREG)]
            mdex = [cpool.tile([128, 96], bf16, name=f"mdex_{r}") for r in range(NREG)]
            pw8t = [cpool.tile([128, 2, 256], f8, name=f"pw8_{p}") for p in range(2)]
            pwid = [cpool.tile([128, 256], bf16, name=f"pwid_{r}") for r in range(2)]
            for p in range(2):
                nc.gpsimd.dma_start(out=pw8t[p][:], in_=dPW8[p])
                nc.gpsimd.dma_start(out=pwid[p][:], in_=dPWI[p])
            for r in range(NREG):
                nc.gpsimd.dma_start(out=ert[r][:], in_=dER[r])
                nc.gpsimd.dma_start(out=mdex[r][:], in_=dMDEX[r])
            pbt = [cpool.tile([128, 1], f32, name=f"pbt_{m}") for m in range(2)]
            for m in range(2):
                nc.gpsimd.dma_start(out=pbt[m][:], in_=dPB[128 * m:128 * (m + 1), :])
            svt = cpool.tile([NHEADS, 1], f32, name="svt")
            nc.gpsimd.dma_start(out=svt[:], in_=dSV[:, :])
            epsw = cpool.tile([1, 96], bf16, name="epsw")
            nc.gpsimd.memset(epsw[:], 1e-15)
            ones1 = cpool.tile([1, TN], bf16, name="ones1")
            nc.gpsimd.memset(ones1[:], 1.0)

            # ---- persistent activations ----
            pad = [qpool.tile([BLK, PADW, PADW], f8, name=f"pad_{b}") for b in range(NBLK)]
            for b in range(NBLK):
                # zero only the 2-wide borders; interior is fully written by
                # pass 1 (keeps these memsets off pass 1's dependency chain)
                nc.gpsimd.memset(pad[b][:, 0:2, :], 0.0)
                nc.gpsimd.memset(pad[b][:, PADW - 2:PADW, :], 0.0)
                nc.gpsimd.memset(pad[b][:, 2:PADW - 2, 0:2], 0.0)
                nc.gpsimd.memset(pad[b][:, 2:PADW - 2, PADW - 2:PADW], 0.0)
            Q = [qpool.tile([128, N], bf16, name=f"Q_{r}") for r in range(NREG)]
            kpart = [qpool.tile([128, NT], f32, name=f"kpart_{r}") for r in range(NREG)]
            vks_sb = [qpool.tile([128, 128], bf16, name=f"vks_{r}") for r in range(NREG)]

            vks_acc = [qpool.tile([128, 128], f32, name=f"vka_{r}")
                       for r in range(NREG)]

            # PSUM pool A: phases 1 + conv (reclaimed before apply phase)
            _pA = tc.tile_pool(name="psumA", bufs=2, space="PSUM")
            ppool = _pA.__enter__()

            # ============ shared per-tile attention stage ====================
            def process_stage(s_idx, nt, ks, vs):
                # first two id-stages: PE transposes (PE is idle pre-conv and
                # this keeps DMA-transfer latency off the early critical path)
                pe_tr = False
                """ks/vs: 2 bf16 [128,512] stage tiles (relu'd k / raw v)."""
                for t in range(2):
                    r = 2 * s_idx + t
                    nc.vector.reduce_sum(out=kpart[r][:, nt:nt + 1], in_=ks[t][:], axis=AX.X)
                for jj in range(4):
                    for t in range(2):
                        kT = spool.tile([128, 128], bf16, name="kT", tag="kT", bufs=12)
                        vT = spool.tile([128, 128], bf16, name="vT", tag="vT", bufs=12)
                        if pe_tr:
                            tp = ppool.tile([128, 128], bf16, name="tp", tag="mm",
                                            bufs=2)
                            nc.tensor.transpose(
                                tp[:], ks[t][:, 128 * jj:128 * (jj + 1)], idt[:])
                            nc.scalar.copy(out=kT[:], in_=tp[:])
                            tp2 = ppool.tile([128, 128], bf16, name="tp2", tag="mm",
                                             bufs=2)
                            nc.tensor.transpose(
                                tp2[:], vs[t][:, 128 * jj:128 * (jj + 1)], idt[:])
                            nc.vector.tensor_copy(out=vT[:], in_=tp2[:])
                        else:
                            nc.sync.dma_start_transpose(
                                out=kT[:], in_=ks[t][:, 128 * jj:128 * (jj + 1)])
                            nc.sync.dma_start_transpose(
                                out=vT[:], in_=vs[t][:, 128 * jj:128 * (jj + 1)])
                        nc.tensor.matmul(vkps[t][:], kT[:], vT[:],
                                         start=(jj == 0), stop=(jj == 3))
                for t in range(2):
                    r = 2 * s_idx + t
                    if nt == 0:
                        nc.vector.tensor_copy(out=vks_acc[r][:], in_=vkps[t][:])
                    else:
                        nc.vector.tensor_tensor(out=vks_acc[r][:], in0=vks_acc[r][:],
                                                in1=vkps[t][:], op=ALU.add)
                    if nt == NT - 1:
                        nc.gpsimd.tensor_copy(out=vks_sb[r][:], in_=vks_acc[r][:])

            # ===== pass 1+2 per nt: stream x, build pad image + id stage =====

            def emit_pass12(nt):
                xt = [spool.tile([128, TN], f32, name="xt", tag=f"xt{k}", bufs=2)
                      for k in range(2)]
                xb = [spool.tile([128, TN], bf16, name="xb", tag=f"xb{k}", bufs=2)
                      for k in range(2)]
                xl = [spool.tile([128, TN], bf16, name="xl", tag=f"xl{k}", bufs=2)
                      for k in range(2)]
                xf8 = spool.tile([128, 2, TN], f8, name="xf8", tag="xf8", bufs=2)
                # xf8 first: it alone gates pass-1, the first PE work per
                # tile; tile 0 converts on the (startup-idle) DVE instead
                cvt = nc.vector if nt == 0 else nc.gpsimd
                for k in range(2):
                    nc.sync.dma_start(out=xt[k][:],
                                      in_=x_in[128 * k:128 * (k + 1), TN * nt:TN * (nt + 1)])
                    cvt.tensor_copy(out=xf8[:, k, :], in_=xt[k][:])
                for k in range(2):
                    cvt.tensor_copy(out=xb[k][:], in_=xt[k][:])
                    cvt.tensor_tensor(out=xl[k][:], in0=xt[k][:], in1=xb[k][:],
                                      op=ALU.subtract)
                # pass 1 (fp8 DoubleRow): natural order -> padded fp8 image;
                # psum is 32*qkv, pad stores PS*qkv -> drain scale 0.5
                for b in range(NBLK):
                    ps = ppool.tile([BLK, 8, WW], f32, name="ps1", tag="mm", bufs=2)
                    nc.tensor.matmul(ps[:].rearrange("p a c -> p (a c)"),
                                     w18t[:, b], xf8[:],
                                     start=True, stop=True,
                                     perf_mode=PM.DoubleRow)
                    dst = pad[b][:, 2 + 8 * nt:10 + 8 * nt, 2:2 + WW]
                    if b % 2 == 0:
                        nc.scalar.activation(out=dst, in_=ps[:], func=AF.Copy,
                                             bias=0.0, scale=PS / 32.0)
                    else:
                        nc.vector.tensor_scalar(out=dst, in0=ps[:],
                                                scalar1=PS / 32.0, scalar2=None,
                                                op0=ALU.mult)
                # pass 2: separated order (fp32r) -> Q + id-scale k/v stages
                ks, vs = [None, None], [None, None]
                for j in range(6):
                    ps = ppool.tile([128, TN], f32, name="ps2", tag="mm", bufs=2)
                    if j < 2:
                        for k in range(2):
                            sl_w = slice(128 * j, 128 * (j + 1))
                            nc.tensor.matmul(ps[:], w2qh[k][:, sl_w], xb[k][:],
                                             start=(k == 0), stop=False)
                            nc.tensor.matmul(ps[:], w2qh[k][:, sl_w], xl[k][:],
                                             start=False, stop=False)
                            nc.tensor.matmul(ps[:], w2ql[k][:, sl_w], xb[k][:],
                                             start=False, stop=(k == 1))
                    else:
                        for k in range(2):
                            nc.tensor.matmul(
                                ps[:], w2kv[k][:, 128 * (j - 2):128 * (j - 1)],
                                xb[k][:], start=(k == 0), stop=(k == 1))
                    if j < 2:
                        nc.scalar.activation(out=Q[j][:, TN * nt:TN * (nt + 1)], in_=ps[:],
                                             func=AF.Relu, bias=bi2[j][:], scale=1.0)
                    elif j < 4:
                        t = j - 2
                        kst = spool.tile([128, TN], bf16, name="ks", tag=f"ks{t}", bufs=4)
                        nc.scalar.activation(out=kst[:], in_=ps[:], func=AF.Relu,
                                             bias=bi2[j][:], scale=1.0)
                        ks[t] = kst
                    else:
                        t = j - 4
                        vst = spool.tile([128, TN], bf16, name="vs", tag=f"vs{t}", bufs=4)
                        nc.vector.tensor_scalar(out=vst[:], in0=ps[:], scalar1=bi2[j][:],
                                                scalar2=None, op0=ALU.add)
                        vs[t] = vst
                return ks, vs

            # ================= fused conv scales (fp8 DoubleRow) =============
            # tap pair u = row-major taps (2u, 2u+1); pair delta in the padded
            # image is off(2u+1)-off(2u); odd tail pairs (tap, tap) with
            # zeroed slot-1 weights (stride 0).
            def pair_deltas(taps):
                ds = []
                for u in range((len(taps) + 1) // 2):
                    if 2 * u + 1 < len(taps):
                        dy0, dx0 = taps[2 * u]
                        dy1, dx1 = taps[2 * u + 1]
                        ds.append((dy1 - dy0) * PADW + (dx1 - dx0))
                    else:
                        ds.append(0)
                return ds

            DELTAS = {3: pair_deltas(TAPS3), 5: pair_deltas(TAPS5)}

            def emit_conv_nt(nt):
                """Both conv scales for one spatial tile, all 8 blocks, then
                their attention stages."""
                stg = {}
                for s_idx in (1, 2):
                    for t in range(2):
                        stg[("k", s_idx, t)] = spool.tile(
                            [128, TN], bf16, name="ks", tag=f"ks{t}", bufs=4)
                        stg[("v", s_idx, t)] = spool.tile(
                            [128, TN], bf16, name="vs", tag=f"vs{t}", bufs=4)
                for b in range(NBLK):
                    for s, s_idx in ((3, 1), (5, 2)):
                        taps, U = TAPS3 if s == 3 else TAPS5, NU[s]
                        cp = ppool.tile([BLK, 8, WW], f32, name="cp", tag="conv",
                                        bufs=4)
                        # per image row: CoreSim's DoubleRow path needs the
                        # rhs to view as exactly [p, 2, N]
                        for u in range(U):
                            dy0, dx0 = taps[2 * u]
                            for r in range(8):
                                rhs = pad[b][:, 2 + 8 * nt + dy0 + r,
                                             2 + dx0:2 + dx0 + WW].copy()
                                rhs.ap.insert(1, [DELTAS[s][u], 2])
                                nc.tensor.matmul(cp[:, r], cw[s][b][:, u], rhs,
                                                 start=(u == 0 and r == 0),
                                                 stop=(u == U - 1 and r == 7),
                                                 perf_mode=PM.DoubleRow)
                        qt, qr = (256 * s_idx + 32 * b) // 128, (32 * b) % 128
                        t2, r2 = b // 4, (32 * b) % 128
                        # drains stay in psum scale (PS*WS); q/k scales cancel
                        # in the attention ratio, v scale folds into rcb
                        nc.scalar.activation(
                            out=Q[qt][qr:qr + 32, TN * nt:TN * (nt + 1)],
                            in_=cp[0:32].rearrange("p a c -> p (a c)"),
                            func=AF.Relu, bias=bc[s][b][0:32, :], scale=1.0)
                        if b >= 7:
                            nc.scalar.activation(
                                out=stg[("k", s_idx, t2)][r2:r2 + 32, :],
                                in_=cp[32:64].rearrange("p a c -> p (a c)"),
                                func=AF.Relu, bias=bc[s][b][32:64, :], scale=1.0)
                        else:
                            nc.vector.tensor_scalar(
                                out=stg[("k", s_idx, t2)][r2:r2 + 32, :],
                                in0=cp[32:64].rearrange("p a c -> p (a c)"),
                                scalar1=bc[s][b][32:64, :], scalar2=0.0,
                                op0=ALU.add, op1=ALU.max)
                        if b % 2 == 0:
                            nc.scalar.activation(
                                out=stg[("v", s_idx, t2)][r2:r2 + 32, :],
                                in_=cp[64:96].rearrange("p a c -> p (a c)"),
                                func=AF.Identity, bias=bc[s][b][64:96, :],
                                scale=1.0)
                        else:
                            nc.vector.tensor_scalar(
                                out=stg[("v", s_idx, t2)][r2:r2 + 32, :],
                                in0=cp[64:96].rearrange("p a c -> p (a c)"),
                                scalar1=bc[s][b][64:96, :], scalar2=None,
                                op0=ALU.add)
                for s_idx in (1, 2):
                    process_stage(s_idx, nt,
                                  [stg[("k", s_idx, t)] for t in range(2)],
                                  [stg[("v", s_idx, t)] for t in range(2)])
                    if nt == NT - 1 and s_idx == 1:
                        emit_asm(2)
                        emit_asm(3)

            # Stream: pass12(nt) feeds pad rows; conv for nt-1 is ready once
            # pass 1 has written rows through nt (s5 needs dy<=+2).
            prev_stage = None
            for nt in range(NT):
                ksvs = emit_pass12(nt)
                if prev_stage is not None:
                    process_stage(0, nt - 1, *prev_stage)
                prev_stage = ksvs
                if nt >= 1:
                    emit_conv_nt(nt - 1)
            process_stage(0, NT - 1, *prev_stage)
            emit_conv_nt(NT - 1)

            # ====== assemble apply weights + denominator lhsT from vk ========
            appw = []
            denw = []
            for r in range(NREG):
                kf = qpool.tile([128, 1], f32, name=f"kfin_{r}")
                nc.vector.reduce_sum(out=kf[:], in_=kpart[r][:], axis=AX.X)
                dwt = qpool.tile([128, 96], bf16, name=f"denw_{r}")
                nc.gpsimd.tensor_scalar(out=dwt[:], in0=mdex[r][:],
                                        scalar1=kf[:], scalar2=None, op0=ALU.mult)
                denw.append(dwt)
                vks = vks_sb[r]
                aw = qpool.tile([128, 128], bf16, name=f"appw_{r}")
                for half in range(2):
                    nc.gpsimd.tensor_tensor(
                        out=aw[:, 64 * half:64 * (half + 1)].rearrange(
                            "p (d h) -> p d h", h=8),
                        in0=vks[:, 64 * half:64 * (half + 1)].rearrange(
                            "p (h d) -> p d h", d=8),
                        in1=mp[half][:].rearrange("p (d h) -> p d h", h=8),
                        op=ALU.mult)
                appw.append(aw)

            # ====== apply (pre-normalized q) + proj ==========================
            _pA.__exit__(None, None, None)
            _pd.__exit__(None, None, None)
            _pB = tc.tile_pool(name="psumB", bufs=2, space="PSUM")
            ppb = _pB.__enter__()

            def emit_ddp(nt):
                """denominator dd[h, n] = den_h . q~_h(n) + eps, all 96 heads"""
                sl = slice(TN * nt, TN * (nt + 1))
                ddp = ppb.tile([96, TN], f32, name="ddp", tag="dd", bufs=2)
                nc.tensor.matmul(ddp[:], epsw[:], ones1[:], start=True, stop=False)
                for r in range(NREG):
                    nc.tensor.matmul(ddp[:], denw[r][:], Q[r][:, sl],
                                     start=False, stop=(r == NREG - 1))
                return ddp

            ddp_cur = emit_ddp(0)
            for nt in range(NT):
                sl = slice(TN * nt, TN * (nt + 1))
                rc = spool.tile([96, TN], f32, name="rc", tag="rc", bufs=1)
                nc.vector.reciprocal_approx_fast(out=rc[:], in_=ddp_cur[:])
                rcb = spool.tile([96, TN], bf16, name="rcb", tag="rcb", bufs=1)
                # fold the per-head v-scale (1 id / SC conv) into the copy
                nc.scalar.activation(out=rcb[:], in_=rc[:], func=AF.Copy,
                                     bias=0.0, scale=svt[:])
                atid = []
                at2 = [None, None]
                for r in range(NREG):
                    rcx = ppb.tile([128, TN], f32, name="rcx", tag="rcx", bufs=2)
                    nc.tensor.matmul(rcx[:], ert[r][:], rcb[:], start=True, stop=True)
                    rxb = spool.tile([128, TN], bf16, name="rxb", tag="rxb", bufs=6)
                    nc.scalar.copy(out=rxb[:], in_=rcx[:])
                    ap2 = ppb.tile([128, TN], f32, name="ap2", tag="ap2", bufs=2)
                    nc.tensor.matmul(ap2[:], appw[r][:], Q[r][:, sl],
                                     start=True, stop=True)
                    if r < 2:
                        at = spool.tile([128, TN], bf16, name="at", tag="at",
                                        bufs=3)
                        nc.vector.tensor_tensor(out=at[:], in0=ap2[:],
                                                in1=rxb[:], op=ALU.mult)
                        atid.append(at)
                    else:
                        p = (r - 2) // 2
                        if r % 2 == 0:
                            at2[p] = spool.tile([128, 2, TN], f8, name="at2",
                                                tag="at2", bufs=3)
                        nc.vector.tensor_tensor(out=at2[p][:, r % 2], in0=ap2[:],
                                                in1=rxb[:], op=ALU.mult)
                # hoist next tile's denominator matmuls here so the at-mults
                # (DVE) finish while PE runs them; pj then starts unstalled
                ddp_next = emit_ddp(nt + 1) if nt + 1 < NT else None
                for m in range(2):
                    pj = ppb.tile([128, TN], f32, name="pj", tag="pj", bufs=2)
                    for r in range(2):
                        nc.tensor.matmul(pj[:], pwid[r][:, 128 * m:128 * (m + 1)],
                                         atid[r][:], start=(r == 0), stop=False)
                    for p in range(2):
                        nc.tensor.matmul(pj[:], pw8t[p][:, :, 128 * m:128 * (m + 1)],
                                         at2[p][:], start=False, stop=(p == 1),
                                         perf_mode=PM.DoubleRow)
                    ob = spool.tile([128, TN], f32, name="ob", tag="ob", bufs=2)
                    nc.scalar.activation(out=ob[:], in_=pj[:], func=AF.Identity,
                                         bias=pbt[m][:], scale=1.0 / 2048.0)
                    nc.sync.dma_start(
                        out=d_out[128 * m:128 * (m + 1), TN * nt:TN * (nt + 1)], in_=ob[:])
                ddp_cur = ddp_next
            _pB.__exit__(None, None, None)
    return nc


def _get_nc():
    if "nc" not in _cache:
        nc = _build()
        nc.compile()
        _cache["nc"] = nc
    return _cache["nc"]


def _feeds(inputs):
    import ml_dtypes

    def bf(a):
        return np.asarray(a, np.float32).astype(ml_dtypes.bfloat16)

    d = _host_weights(inputs)
    base = {
        "w2qh": bf(np.ascontiguousarray(d["w2t"][:, :256])),
        "w2ql": bf(np.ascontiguousarray(d["w2t"][:, :256])
                   - np.asarray(bf(np.ascontiguousarray(d["w2t"][:, :256])),
                                np.float32)),
        "bi2": d["bi2"].astype(np.float32),
        "fw3": d["fw3"].astype(ml_dtypes.float8_e4m3),
        "fw5": d["fw5"].astype(ml_dtypes.float8_e4m3),
        "bc3": d["bc3"].astype(np.float32), "bc5": d["bc5"].astype(np.float32),
        "idt": bf(d["idt"]),
        "mp0": bf(d["mp0"]), "mp1": bf(d["mp1"]),
        "er": bf(d["er"]), "mdex": bf(d["mdex"]),
        "pw8": d["pw8"].astype(ml_dtypes.float8_e4m3),
        "pwid": bf(d["pwid"]),
        "pb": d["pb"].astype(np.float32),
        "svec": d["svec"].astype(np.float32),
        "w18": d["w18"].astype(ml_dtypes.float8_e4m3),
        "w2kv": bf(np.ascontiguousarray(d["w2t"][:, 256:])),
    }
    x = np.asarray(inputs["x"], np.float32).reshape(B, CIN, N)
    return base, x


def kernel(**inputs):
    from concourse.bass_utils import run_bass_kernel_spmd

    base, x = _feeds(inputs)
    in_maps = []
    for c in range(B):
        m = dict(base)
        m["xf"] = np.ascontiguousarray(x[c])
        in_maps.append(m)
    nc = _get_nc()
    res = run_bass_kernel_spmd(nc, in_maps, list(range(B))).results
    out = np.stack([np.asarray(r["out"]).reshape(CIN, HH, WW) for r in res])
    return out.astype(np.float32)



# revision 66
# speedup vs baseline: 1.1271x; 1.0143x over previous
"""LiteMLA (EfficientViT multi-scale linear attention) Trainium2 Bass kernel.

Sharding: data-parallel over batch B=8 across 8 NeuronCores (1 image/core).
Per-core pipeline:
  1. Streamed per-nt x: pass1 (bf16, natural channel order) -> zero-padded
     SBUF image for conv taps; pass2 -> attention Q buffer + id-scale K/V
     stages. The q channels use Dekker-split bf16 matmuls (whi.xhi +
     whi.xlo + wlo.xhi, ~16-bit effective mantissa): the id-scale heads are
     ill-conditioned (att = (vk@relu q)/(den@relu q) is 0/0 at positions
     where all 8 q dims are negative), so relu(q)'s sign pattern must track
     the fp32 reference closely; plain bf16 or HW-float32r inputs flip
     signs and cost ~0.15 rel err. k/v stay bf16 (4096-term averages).
     s3 conv is emitted interleaved into this loop to fill PE idle.
  2. s3/s5: depthwise 3x3/5x5 + grouped 1x1 FUSED on host into per-tap
     block-diagonal [96,96] weights; all taps of a block fetched in ONE
     sync-queue DMA; PE matmuls accumulate taps in PSUM reading shifted
     slices of the padded image.
  3. relu-linear attention: per spatial tile, relu(k)/v transposed by the
     DMA engines (xbar dma_start_transpose, contiguous [128,128] dst tiles)
     and reduced into per-16-head vk outer products; per-tile PSUM partials
     fold into SBUF accumulators on DVE; denominator = row-sums of relu(k).
  4. apply: att_raw = vk @ relu(q) via block-diagonal apply weights on raw
     Q (no per-position pre-scale, keeping PE free of the normalize chain);
     the batched [96,TN] denominator reciprocal is broadcast-expanded per
     region on PE and applied at the PSUM drain (one DVE multiply); proj
     contracts 128 rows (2 groups per matmul). Each tile's attention stage
     is processed one iteration deferred so its DMA-transpose issues overlap
     the next tile's matmuls.

All SBUF/PSUM operand slices start at partition 0/32/64/96 (HW requirement).
"""

import sys
import numpy as np

sys.path.insert(0, "/opt/trn_rl_repo")

B, CIN, HH, WW = 8, 256, 64, 64
N = HH * WW            # 4096
HEADS = 32             # per scale
C3 = 768
NHEADS = 96
PADW = WW + 4          # 68
NT = 8                 # spatial tiles of 512 positions (8 image rows each)
TN = 512
HALF = 2               # nts processed per conv weight fetch
TAPS3 = [(dy, dx) for dy in (-1, 0, 1) for dx in (-1, 0, 1)]
TAPS5 = [(dy, dx) for dy in (-2, -1, 0, 1, 2) for dx in (-2, -1, 0, 1, 2)]
NBLK = 8               # conv channel blocks of 4 head-groups
BLK = 96
NREG = 6               # vk regions of 16 heads

_cache = {}

PS = 16.0              # fp8 pad image pre-scale
WS = 256.0             # fp8 fused conv weight pre-scale
SC = 1.0 / (PS * WS)   # conv psum drain scale (2^-12)
NU = {3: 5, 5: 13}     # DoubleRow tap-pair units per scale


def _head_of(g12, i):
    return 16 * (g12 // 2) + 8 * (g12 % 2) + i


def _host_weights(inp):
    f32 = np.float32
    W = np.asarray(inp["qkv_w"], f32)[:, :, 0, 0]            # [768, 256]
    qkv_b = np.asarray(inp["qkv_b"], f32)
    pw = {3: np.asarray(inp["pw3_w"], f32)[:, :, 0, 0],
          5: np.asarray(inp["pw5_w"], f32)[:, :, 0, 0]}
    pwb = {3: np.asarray(inp["pw3_b"], f32), 5: np.asarray(inp["pw5_b"], f32)}
    dw = {3: np.asarray(inp["dw3_w"], f32)[:, 0],
          5: np.asarray(inp["dw5_w"], f32)[:, 0]}
    dwb = {3: np.asarray(inp["dw3_b"], f32), 5: np.asarray(inp["dw5_b"], f32)}
    proj_w = np.asarray(inp["proj_w"], f32)[:, :, 0, 0]      # [256, 768]
    proj_b = np.asarray(inp["proj_b"], f32)

    d = {}
    # pass1 weights pre-scaled by PS: pad fp8 image = PS*qkv via a pure-copy
    # drain (no scale op needed)
    d["w1t"] = np.ascontiguousarray(W.T) * PS                # [256, 768]
    # per-head reciprocal scale: attention is scale-invariant in q/k per head
    # and scales linearly with v, so conv drains skip the 1/(PS*WS) descale
    # and the v-scale is folded into the rcb copy (heads 32..95 are conv)
    sv = np.ones((NHEADS, 1), f32)
    sv[32:] = 1.0 / (PS * WS)
    # conv att pre-scaled 512x so its fp8 att tiles (DoubleRow proj) stay in
    # e4m3 normal range; id att stays bf16 (dominates output magnitude)
    sv[32:] *= 512.0
    d["svec"] = sv
    # pass1 fp8 DoubleRow weights: [NBLK, 128, 2, 96], *32 (drain scale .5*PS/16)
    w18 = np.empty((NBLK, 128, 2, BLK), f32)
    for b in range(NBLK):
        for j in range(2):
            w18[b, :, j, :] = 32.0 * W[BLK * b:BLK * (b + 1), 128 * j:128 * (j + 1)].T
    d["w18"] = w18
    perm2 = np.empty(768, np.int64)
    for h in range(HEADS):
        for e in range(8):
            perm2[h * 8 + e] = h * 24 + e
            perm2[256 + h * 8 + e] = h * 24 + 8 + e
            perm2[512 + h * 8 + e] = h * 24 + 16 + e
    d["w2t"] = np.ascontiguousarray(W[perm2].T)
    d["bi2"] = qkv_b[perm2].reshape(768, 1)

    # fused conv weights: per tap, 8 blocks of 4 groups, [96in, 96out q|k|v];
    # stored per-block contiguous over taps: [NBLK, 96in, taps*96out]
    for s, taps in ((3, TAPS3), (5, TAPS5)):
        fw = np.zeros((len(taps), NBLK, BLK, BLK), f32)
        fb = np.zeros((NBLK, BLK), f32)
        for b in range(NBLK):
            for gl in range(4):
                g = 4 * b + gl
                M24 = pw[s][g * 24:(g + 1) * 24]             # [24 out(oo), 24 in]
                bias24 = pwb[s][g * 24:(g + 1) * 24] + M24 @ dwb[s][g * 24:(g + 1) * 24]
                for oo in range(24):
                    m = (oo // 8) * 32 + gl * 8 + (oo % 8)   # [q32|k32|v32]
                    fb[b, m] = bias24[oo]
                dvec = dw[s][g * 24:(g + 1) * 24]            # [24 in, kh, kw]
                for ti, (dy, dx) in enumerate(taps):
                    wt = M24 * dvec[:, dy + s // 2, dx + s // 2][None, :]
                    for oo in range(24):
                        m = (oo // 8) * 32 + gl * 8 + (oo % 8)
                        fw[ti, b, gl * 24:(gl + 1) * 24, m] = wt[oo]
        # fp8 DoubleRow pair-stacked: [NBLK, BLK_in, U, 2, BLK_out]; unit u
        # holds taps (2u, 2u+1); odd tap count -> last unit slot 1 zeroed
        U = NU[s]
        fw8 = np.zeros((NBLK, BLK, U, 2, BLK), f32)
        for u in range(U):
            for j in range(2):
                ti = 2 * u + j
                if ti < len(taps):
                    fw8[:, :, u, j, :] = fw[ti] * WS
        d[f"fw{s}"] = fw8
        # conv drain bias in raw-psum units (outputs stay scaled by PS*WS)
        d[f"bc{s}"] = fb.reshape(NBLK, BLK, 1) * (PS * WS)

    d["idt"] = np.eye(128, dtype=f32)

    # masks for vk -> apply-weight assembly (dd-major cols, no den col)
    for half in range(2):
        mp = np.zeros((128, 64), f32)   # [(hp,e), (dd,h)]
        for p in range(128):
            hp = p // 8
            for h in range(8):
                if hp == h + 8 * half:
                    for dd in range(8):
                        mp[p, 8 * dd + h] = 1.0
        d[f"mp{half}"] = mp

    # expand matrices: er maps rc rows (96 heads) onto ap2's row layout
    # (col p: half=p//64, dd, i -> head 16r+8*(p//64)+p%8); mdex places den
    # values for the Q-region row layout (row p -> head 16r + p//8)
    er = np.zeros((NREG, 96, 128), f32)
    mdex = np.zeros((NREG, 128, 96), f32)
    for r in range(NREG):
        for p in range(128):
            er[r, 16 * r + 8 * (p // 64) + p % 8, p] = 1.0
            mdex[r, p, 16 * r + p // 8] = 1.0
    d["er"] = er
    d["mdex"] = mdex

    # proj lhsT [6, 128, 256]: rows 0:64 group 2r, 64:128 group 2r+1; row
    # (64*half + 8*dd + i) = proj col of head _head_of(2r+half, i), dim dd
    PW2 = np.zeros((NREG, 128, 256), f32)
    for r in range(NREG):
        for half in range(2):
            for i in range(8):
                Hh = _head_of(2 * r + half, i)
                for dd in range(8):
                    PW2[r, 64 * half + 8 * dd + i] = proj_w[:, 8 * Hh + dd]
    # proj: id regions 0,1 in bf16 at psum scale WP; conv region pairs
    # (2,3),(4,5) as fp8 DoubleRow k-tiles at WP/512 (att carries the 512)
    WP = 2048.0
    d["pwid"] = WP * PW2[0:2]
    pw8 = np.empty((2, 128, 2, 256), f32)
    for p in range(2):
        for j in range(2):
            pw8[p, :, j, :] = (WP / 512.0) * PW2[2 + 2 * p + j]
    d["pw8"] = pw8
    d["pb"] = proj_b.reshape(256, 1)
    return d


def _build():
    import concourse.bass as bass
    import concourse.bacc as bacc_mod
    import concourse.mybir as mybir
    from concourse.tile import TileContext

    dt = mybir.dt
    f32, bf16, f8 = dt.float32, dt.bfloat16, dt.float8e4
    AF = mybir.ActivationFunctionType
    ALU = mybir.AluOpType
    AX = mybir.AxisListType
    PM = mybir.MatmulPerfMode

    nc = bacc_mod.Bacc()
    x_in = nc.dram_tensor("xf", [CIN, N], f32, kind="ExternalInput")
    dW2QH = nc.dram_tensor("w2qh", [CIN, 256], bf16, kind="ExternalInput")
    dW2QL = nc.dram_tensor("w2ql", [CIN, 256], bf16, kind="ExternalInput")
    dBI2 = nc.dram_tensor("bi2", [C3, 1], f32, kind="ExternalInput")
    dFW3 = nc.dram_tensor("fw3", [NBLK, BLK, NU[3], 2, BLK], f8, kind="ExternalInput")
    dFW5 = nc.dram_tensor("fw5", [NBLK, BLK, NU[5], 2, BLK], f8, kind="ExternalInput")
    dBC3 = nc.dram_tensor("bc3", [NBLK, BLK, 1], f32, kind="ExternalInput")
    dBC5 = nc.dram_tensor("bc5", [NBLK, BLK, 1], f32, kind="ExternalInput")
    dIDT = nc.dram_tensor("idt", [128, 128], bf16, kind="ExternalInput")
    dMP = [nc.dram_tensor(f"mp{h}", [128, 64], bf16, kind="ExternalInput") for h in range(2)]
    dER = nc.dram_tensor("er", [NREG, 96, 128], bf16, kind="ExternalInput")
    dMDEX = nc.dram_tensor("mdex", [NREG, 128, 96], bf16, kind="ExternalInput")
    dPW8 = nc.dram_tensor("pw8", [2, 128, 2, 256], f8, kind="ExternalInput")
    dPWI = nc.dram_tensor("pwid", [2, 128, 256], bf16, kind="ExternalInput")
    dPB = nc.dram_tensor("pb", [256, 1], f32, kind="ExternalInput")
    dSV = nc.dram_tensor("svec", [NHEADS, 1], f32, kind="ExternalInput")
    dW18 = nc.dram_tensor("w18", [NBLK, 128, 2, BLK], f8, kind="ExternalInput")
    dW2kv = nc.dram_tensor("w2kv", [CIN, 512], bf16, kind="ExternalInput")
    d_out = nc.dram_tensor("out", [CIN, N], f32, kind="ExternalOutput")

    with TileContext(nc) as tc:
        with (
            tc.tile_pool(name="consts", bufs=1) as cpool,
            tc.tile_pool(name="persist", bufs=1) as qpool,
            tc.tile_pool(name="wstream", bufs=2) as wpool,
            tc.tile_pool(name="stage", bufs=2) as spool,
        ):
            # ---- constants (off the sync queue so x DMAs start immediately) --
            w2qh = [cpool.tile([128, 256], bf16, name=f"w2qh_{k}") for k in range(2)]
            w2ql = [cpool.tile([128, 256], bf16, name=f"w2ql_{k}") for k in range(2)]
            for k in range(2):
                nc.scalar.dma_start(out=w2qh[k][:], in_=dW2QH[128 * k:128 * (k + 1), :])
                nc.scalar.dma_start(out=w2ql[k][:], in_=dW2QL[128 * k:128 * (k + 1), :])
            bi2 = [cpool.tile([128, 1], f32, name=f"bi2_{j}") for j in range(6)]
            for j in range(6):
                nc.scalar.dma_start(out=bi2[j][:], in_=dBI2[128 * j:128 * (j + 1), :])
            # pass-1/2 weights FIRST: they gate the very first PE work;
            # the bulky conv weights follow (not needed until conv(0) ~15us in)
            w18t = cpool.tile([128, NBLK, 2, BLK], f8, name="w18t")
            for b in range(NBLK):
                nc.scalar.dma_start(out=w18t[:, b], in_=dW18[b])
            w2kv = [cpool.tile([128, 512], bf16, name=f"w2kv_{k}") for k in range(2)]
            for k in range(2):
                nc.scalar.dma_start(out=w2kv[k][:], in_=dW2kv[128 * k:128 * (k + 1), :])
            bc = {}
            for s, db in ((3, dBC3), (5, dBC5)):
                bc[s] = [cpool.tile([BLK, 1], f32, name=f"bc{s}_{b}") for b in range(NBLK)]
                for b in range(NBLK):
                    nc.gpsimd.dma_start(out=bc[s][b][:], in_=db[b])
            # resident fp8 DoubleRow conv weights: [96, U, 2, 96] per block
            cw = {}
            for s, dfw in ((3, dFW3), (5, dFW5)):
                cw[s] = [cpool.tile([BLK, NU[s], 2, BLK], f8, name=f"cw{s}_{b}")
                         for b in range(NBLK)]
                for b in range(NBLK):
                    eng = nc.scalar if b % 2 == 0 else nc.gpsimd
                    eng.dma_start(out=cw[s][b][:], in_=dfw[b])
            idt = cpool.tile([128, 128], bf16, name="idt")
            nc.scalar.dma_start(out=idt[:], in_=dIDT[:, :])
            mp = [cpool.tile([128, 64], bf16, name=f"mp_{h}") for h in range(2)]
            for h in range(2):
                nc.gpsimd.dma_start(out=mp[h][:], in_=dMP[h][:, :])
            ert = [cpool.tile([96, 128], bf16, name=f"er_{r}") for r in range(NREG)]
            mdex = [cpool.tile([128, 96], bf16, name=f"mdex_{r}") for r in range(NREG)]
            pw8t = [cpool.tile([128, 2, 256], f8, name=f"pw8_{p}") for p in range(2)]
            pwid = [cpool.tile([128, 256], bf16, name=f"pwid_{r}") for r in range(2)]
            for p in range(2):
                nc.gpsimd.dma_start(out=pw8t[p][:], in_=dPW8[p])
                nc.gpsimd.dma_start(out=pwid[p][:], in_=dPWI[p])
            for r in range(NREG):
                nc.gpsimd.dma_start(out=ert[r][:], in_=dER[r])
                nc.gpsimd.dma_start(out=mdex[r][:], in_=dMDEX[r])
            pbt = [cpool.tile([128, 1], f32, name=f"pbt_{m}") for m in range(2)]
            for m in range(2):
                nc.gpsimd.dma_start(out=pbt[m][:], in_=dPB[128 * m:128 * (m + 1), :])
            svt = cpool.tile([NHEADS, 1], f32, name="svt")
            nc.gpsimd.dma_start(out=svt[:], in_=dSV[:, :])
            epsw = cpool.tile([1, 96], bf16, name="epsw")
            nc.gpsimd.memset(epsw[:], 1e-15)
            ones1 = cpool.tile([1, TN], bf16, name="ones1")
            nc.gpsimd.memset(ones1[:], 1.0)

            # ---- persistent activations ----
            pad = [qpool.tile([BLK, PADW, PADW], f8, name=f"pad_{b}") for b in range(NBLK)]
            for b in range(NBLK):
                # zero only the 2-wide borders; interior is fully written by
                # pass 1 (keeps these memsets off pass 1's dependency chain)
                nc.gpsimd.memset(pad[b][:, 0:2, :], 0.0)
                nc.gpsimd.memset(pad[b][:, PADW - 2:PADW, :], 0.0)
                nc.gpsimd.memset(pad[b][:, 2:PADW - 2, 0:2], 0.0)
                nc.gpsimd.memset(pad[b][:, 2:PADW - 2, PADW - 2:PADW], 0.0)
            Q = [qpool.tile([128, N], bf16, name=f"Q_{r}") for r in range(NREG)]
            kpart = [qpool.tile([128, NT], f32, name=f"kpart_{r}") for r in range(NREG)]
            vks_sb = [qpool.tile([128, 128], bf16, name=f"vks_{r}") for r in range(NREG)]

            vks_acc = [qpool.tile([128, 128], f32, name=f"vka_{r}")
                       for r in range(NREG)]

            # PSUM pool A: phases 1 + conv (reclaimed before apply phase)
            _pA = tc.tile_pool(name="psumA", bufs=2, space="PSUM")
            ppool = _pA.__enter__()

            # ============ shared per-tile attention stage ====================
            def process_stage(s_idx, nt, ks, vs):
                # first two id-stages: PE transposes (PE is idle pre-conv and
                # this keeps DMA-transfer latency off the early critical path)
                pe_tr = False
                """ks/vs: 2 bf16 [128,512] stage tiles (relu'd k / raw v)."""
                for t in range(2):
                    r = 2 * s_idx + t
                    nc.vector.reduce_sum(out=kpart[r][:, nt:nt + 1], in_=ks[t][:], axis=AX.X)
                for jj in range(4):
                    for t in range(2):
                        kT = spool.tile([128, 128], bf16, name="kT", tag="kT", bufs=12)
                        vT = spool.tile([128, 128], bf16, name="vT", tag="vT", bufs=12)
                        if pe_tr:
                            tp = ppool.tile([128, 128], bf16, name="tp", tag="mm",
                                            bufs=2)
                            nc.tensor.transpose(
                                tp[:], ks[t][:, 128 * jj:128 * (jj + 1)], idt[:])
                            nc.scalar.copy(out=kT[:], in_=tp[:])
                            tp2 = ppool.tile([128, 128], bf16, name="tp2", tag="mm",
                                             bufs=2)
                            nc.tensor.transpose(
                                tp2[:], vs[t][:, 128 * jj:128 * (jj + 1)], idt[:])
                            nc.vector.tensor_copy(out=vT[:], in_=tp2[:])
                        else:
                            nc.sync.dma_start_transpose(
                                out=kT[:], in_=ks[t][:, 128 * jj:128 * (jj + 1)])
                            nc.sync.dma_start_transpose(
                                out=vT[:], in_=vs[t][:, 128 * jj:128 * (jj + 1)])
                        nc.tensor.matmul(vkps[t][:], kT[:], vT[:],
                                         start=(jj == 0), stop=(jj == 3))
                for t in range(2):
                    r = 2 * s_idx + t
                    if nt == 0:
                        nc.vector.tensor_copy(out=vks_acc[r][:], in_=vkps[t][:])
                    else:
                        nc.vector.tensor_tensor(out=vks_acc[r][:], in0=vks_acc[r][:],
                                                in1=vkps[t][:], op=ALU.add)
                    if nt == NT - 1:
                        nc.gpsimd.tensor_copy(out=vks_sb[r][:], in_=vks_acc[r][:])

            # ===== pass 1+2 per nt: stream x, build pad image + id stage =====

            def emit_pass12(nt):
                xt = [spool.tile([128, TN], f32, name="xt", tag=f"xt{k}", bufs=2)
                      for k in range(2)]
                xb = [spool.tile([128, TN], bf16, name="xb", tag=f"xb{k}", bufs=2)
                      for k in range(2)]
                xl = [spool.tile([128, TN], bf16, name="xl", tag=f"xl{k}", bufs=2)
                      for k in range(2)]
                xf8 = spool.tile([128, 2, TN], f8, name="xf8", tag="xf8", bufs=2)
                # xf8 first: it alone gates pass-1, the first PE work per
                # tile; tile 0 converts on the (startup-idle) DVE instead
                cvt = nc.vector if nt == 0 else nc.gpsimd
                for k in range(2):
                    nc.sync.dma_start(out=xt[k][:],
                                      in_=x_in[128 * k:128 * (k + 1), TN * nt:TN * (nt + 1)])
                    cvt.tensor_copy(out=xf8[:, k, :], in_=xt[k][:])
                for k in range(2):
                    cvt.tensor_copy(out=xb[k][:], in_=xt[k][:])
                    cvt.tensor_tensor(out=xl[k][:], in0=xt[k][:], in1=xb[k][:],
                                      op=ALU.subtract)
                # pass 1 (fp8 DoubleRow): natural order -> padded fp8 image;
                # psum is 32*qkv, pad stores PS*qkv -> drain scale 0.5
                for b in range(NBLK):
                    ps = ppool.tile([BLK, 8, WW], f32, name="ps1", tag="mm", bufs=2)
                    nc.tensor.matmul(ps[:].rearrange("p a c -> p (a c)"),
                                     w18t[:, b], xf8[:],
                                     start=True, stop=True,
                                     perf_mode=PM.DoubleRow)
                    dst = pad[b][:, 2 + 8 * nt:10 + 8 * nt, 2:2 + WW]
                    if b % 2 == 0:
                        nc.scalar.activation(out=dst, in_=ps[:], func=AF.Copy,
                                             bias=0.0, scale=PS / 32.0)
                    else:
                        nc.vector.tensor_scalar(out=dst, in0=ps[:],
                                                scalar1=PS / 32.0, scalar2=None,
                                                op0=ALU.mult)
                # pass 2: separated order (fp32r) -> Q + id-scale k/v stages
                ks, vs = [None, None], [None, None]
                for j in range(6):
                    ps = ppool.tile([128, TN], f32, name="ps2", tag="mm", bufs=2)
                    if j < 2:
                        for k in range(2):
                            sl_w = slice(128 * j, 128 * (j + 1))
                            nc.tensor.matmul(ps[:], w2qh[k][:, sl_w], xb[k][:],
                                             start=(k == 0), stop=False)
                            nc.tensor.matmul(ps[:], w2qh[k][:, sl_w], xl[k][:],
                                             start=False, stop=False)
                            nc.tensor.matmul(ps[:], w2ql[k][:, sl_w], xb[k][:],
                                             start=False, stop=(k == 1))
                    else:
                        for k in range(2):
                            nc.tensor.matmul(
                                ps[:], w2kv[k][:, 128 * (j - 2):128 * (j - 1)],
                                xb[k][:], start=(k == 0), stop=(k == 1))
                    if j < 2:
                        nc.scalar.activation(out=Q[j][:, TN * nt:TN * (nt + 1)], in_=ps[:],
                                             func=AF.Relu, bias=bi2[j][:], scale=1.0)
                    elif j < 4:
                        t = j - 2
                        kst = spool.tile([128, TN], bf16, name="ks", tag=f"ks{t}", bufs=4)
                        nc.scalar.activation(out=kst[:], in_=ps[:], func=AF.Relu,
                                             bias=bi2[j][:], scale=1.0)
                        ks[t] = kst
                    else:
                        t = j - 4
                        vst = spool.tile([128, TN], bf16, name="vs", tag=f"vs{t}", bufs=4)
                        nc.vector.tensor_scalar(out=vst[:], in0=ps[:], scalar1=bi2[j][:],
                                                scalar2=None, op0=ALU.add)
                        vs[t] = vst
                return ks, vs

            # ================= fused conv scales (fp8 DoubleRow) =============
            # tap pair u = row-major taps (2u, 2u+1); pair delta in the padded
            # image is off(2u+1)-off(2u); odd tail pairs (tap, tap) with
            # zeroed slot-1 weights (stride 0).
            def pair_deltas(taps):
                ds = []
                for u in range((len(taps) + 1) // 2):
                    if 2 * u + 1 < len(taps):
                        dy0, dx0 = taps[2 * u]
                        dy1, dx1 = taps[2 * u + 1]
                        ds.append((dy1 - dy0) * PADW + (dx1 - dx0))
                    else:
                        ds.append(0)
                return ds

            DELTAS = {3: pair_deltas(TAPS3), 5: pair_deltas(TAPS5)}

            def emit_conv_nt(nt):
                """Both conv scales for one spatial tile, all 8 blocks, then
                their attention stages."""
                stg = {}
                for s_idx in (1, 2):
                    for t in range(2):
                        stg[("k", s_idx, t)] = spool.tile(
                            [128, TN], bf16, name="ks", tag=f"ks{t}", bufs=4)
                        stg[("v", s_idx, t)] = spool.tile(
                            [128, TN], bf16, name="vs", tag=f"vs{t}", bufs=4)
                for b in range(NBLK):
                    for s, s_idx in ((3, 1), (5, 2)):
                        taps, U = TAPS3 if s == 3 else TAPS5, NU[s]
                        cp = ppool.tile([BLK, 8, WW], f32, name="cp", tag="conv",
                                        bufs=4)
                        # per image row: CoreSim's DoubleRow path needs the
                        # rhs to view as exactly [p, 2, N]
                        for u in range(U):
                            dy0, dx0 = taps[2 * u]
                            for r in range(8):
                                rhs = pad[b][:, 2 + 8 * nt + dy0 + r,
                                             2 + dx0:2 + dx0 + WW].copy()
                                rhs.ap.insert(1, [DELTAS[s][u], 2])
                                nc.tensor.matmul(cp[:, r], cw[s][b][:, u], rhs,
                                                 start=(u == 0 and r == 0),
                                                 stop=(u == U - 1 and r == 7),
                                                 perf_mode=PM.DoubleRow)
                        qt, qr = (256 * s_idx + 32 * b) // 128, (32 * b) % 128
                        t2, r2 = b // 4, (32 * b) % 128
                        # drains stay in psum scale (PS*WS); q/k scales cancel
                        # in the attention ratio, v scale folds into rcb.
                        # k/v first: they gate the attention stages; q is not
                        # consumed until the apply phase
                        if b >= 7:
                            nc.scalar.activation(
                                out=stg[("k", s_idx, t2)][r2:r2 + 32, :],
                                in_=cp[32:64].rearrange("p a c -> p (a c)"),
                                func=AF.Relu, bias=bc[s][b][32:64, :], scale=1.0)
                        else:
                            nc.vector.tensor_scalar(
                                out=stg[("k", s_idx, t2)][r2:r2 + 32, :],
                                in0=cp[32:64].rearrange("p a c -> p (a c)"),
                                scalar1=bc[s][b][32:64, :], scalar2=0.0,
                                op0=ALU.add, op1=ALU.max)
                        if b % 2 == 0:
                            nc.scalar.activation(
                                out=stg[("v", s_idx, t2)][r2:r2 + 32, :],
                                in_=cp[64:96].rearrange("p a c -> p (a c)"),
                                func=AF.Identity, bias=bc[s][b][64:96, :],
                                scale=1.0)
                        else:
                            nc.vector.tensor_scalar(
                                out=stg[("v", s_idx, t2)][r2:r2 + 32, :],
                                in0=cp[64:96].rearrange("p a c -> p (a c)"),
                                scalar1=bc[s][b][64:96, :], scalar2=None,
                                op0=ALU.add)
                        nc.scalar.activation(
                            out=Q[qt][qr:qr + 32, TN * nt:TN * (nt + 1)],
                            in_=cp[0:32].rearrange("p a c -> p (a c)"),
                            func=AF.Relu, bias=bc[s][b][0:32, :], scale=1.0)
                for s_idx in (1, 2):
                    process_stage(s_idx, nt,
                                  [stg[("k", s_idx, t)] for t in range(2)],
                                  [stg[("v", s_idx, t)] for t in range(2)])
                    if nt == NT - 1 and s_idx == 1:
                        emit_asm(2)
                        emit_asm(3)

            # Stream: pass12(nt) feeds pad rows; conv for nt-1 is ready once
            # pass 1 has written rows through nt (s5 needs dy<=+2).
            prev_stage = None
            for nt in range(NT):
                ksvs = emit_pass12(nt)
                if prev_stage is not None:
                    process_stage(0, nt - 1, *prev_stage)
                prev_stage = ksvs
                if nt >= 1:
                    emit_conv_nt(nt - 1)
            process_stage(0, NT - 1, *prev_stage)
            emit_conv_nt(NT - 1)

            # ====== assemble apply weights + denominator lhsT from vk ========
            appw = []
            denw = []
            for r in range(NREG):
                kf = qpool.tile([128, 1], f32, name=f"kfin_{r}")
                nc.vector.reduce_sum(out=kf[:], in_=kpart[r][:], axis=AX.X)
                dwt = qpool.tile([128, 96], bf16, name=f"denw_{r}")
                nc.gpsimd.tensor_scalar(out=dwt[:], in0=mdex[r][:],
                                        scalar1=kf[:], scalar2=None, op0=ALU.mult)
                denw.append(dwt)
                vks = vks_sb[r]
                aw = qpool.tile([128, 128], bf16, name=f"appw_{r}")
                for half in range(2):
                    nc.gpsimd.tensor_tensor(
                        out=aw[:, 64 * half:64 * (half + 1)].rearrange(
                            "p (d h) -> p d h", h=8),
                        in0=vks[:, 64 * half:64 * (half + 1)].rearrange(
                            "p (h d) -> p d h", d=8),
                        in1=mp[half][:].rearrange("p (d h) -> p d h", h=8),
                        op=ALU.mult)
                appw.append(aw)

            # ====== apply (pre-normalized q) + proj ==========================
            _pA.__exit__(None, None, None)
            _pd.__exit__(None, None, None)
            _pB = tc.tile_pool(name="psumB", bufs=2, space="PSUM")
            ppb = _pB.__enter__()

            def emit_ddp(nt):
                """denominator dd[h, n] = den_h . q~_h(n) + eps, all 96 heads"""
                sl = slice(TN * nt, TN * (nt + 1))
                ddp = ppb.tile([96, TN], f32, name="ddp", tag="dd", bufs=2)
                nc.tensor.matmul(ddp[:], epsw[:], ones1[:], start=True, stop=False)
                for r in range(NREG):
                    nc.tensor.matmul(ddp[:], denw[r][:], Q[r][:, sl],
                                     start=False, stop=(r == NREG - 1))
                return ddp

            ddp_cur = emit_ddp(0)
            for nt in range(NT):
                sl = slice(TN * nt, TN * (nt + 1))
                rc = spool.tile([96, TN], f32, name="rc", tag="rc", bufs=1)
                nc.vector.reciprocal_approx_fast(out=rc[:], in_=ddp_cur[:])
                rcb = spool.tile([96, TN], bf16, name="rcb", tag="rcb", bufs=1)
                # fold the per-head v-scale (1 id / SC conv) into the copy
                nc.scalar.activation(out=rcb[:], in_=rc[:], func=AF.Copy,
                                     bias=0.0, scale=svt[:])
                atid = []
                at2 = [None, None]
                for r in range(NREG):
                    rcx = ppb.tile([128, TN], f32, name="rcx", tag="rcx", bufs=2)
                    nc.tensor.matmul(rcx[:], ert[r][:], rcb[:], start=True, stop=True)
                    rxb = spool.tile([128, TN], bf16, name="rxb", tag="rxb", bufs=6)
                    nc.scalar.copy(out=rxb[:], in_=rcx[:])
                    ap2 = ppb.tile([128, TN], f32, name="ap2", tag="ap2", bufs=2)
                    nc.tensor.matmul(ap2[:], appw[r][:], Q[r][:, sl],
                                     start=True, stop=True)
                    if r < 2:
                        at = spool.tile([128, TN], bf16, name="at", tag="at",
                                        bufs=3)
                        nc.vector.tensor_tensor(out=at[:], in0=ap2[:],
                                                in1=rxb[:], op=ALU.mult)
                        atid.append(at)
                    else:
                        p = (r - 2) // 2
                        if r % 2 == 0:
                            at2[p] = spool.tile([128, 2, TN], f8, name="at2",
                                                tag="at2", bufs=3)
                        nc.vector.tensor_tensor(out=at2[p][:, r % 2], in0=ap2[:],
                                                in1=rxb[:], op=ALU.mult)
                # hoist next tile's denominator matmuls here so the at-mults
                # (DVE) finish while PE runs them; pj then starts unstalled
                ddp_next = emit_ddp(nt + 1) if nt + 1 < NT else None
                for m in range(2):
                    pj = ppb.tile([128, TN], f32, name="pj", tag="pj", bufs=2)
                    for r in range(2):
                        nc.tensor.matmul(pj[:], pwid[r][:, 128 * m:128 * (m + 1)],
                                         atid[r][:], start=(r == 0), stop=False)
                    for p in range(2):
                        nc.tensor.matmul(pj[:], pw8t[p][:, :, 128 * m:128 * (m + 1)],
                                         at2[p][:], start=False, stop=(p == 1),
                                         perf_mode=PM.DoubleRow)
                    ob = spool.tile([128, TN], f32, name="ob", tag="ob", bufs=2)
                    nc.scalar.activation(out=ob[:], in_=pj[:], func=AF.Identity,
                                         bias=pbt[m][:], scale=1.0 / 2048.0)
                    nc.sync.dma_start(
                        out=d_out[128 * m:128 * (m + 1), TN * nt:TN * (nt + 1)], in_=ob[:])
                ddp_cur = ddp_next
            _pB.__exit__(None, None, None)
    return nc


def _get_nc():
    if "nc" not in _cache:
        nc = _build()
        nc.compile()
        _cache["nc"] = nc
    return _cache["nc"]


def _feeds(inputs):
    import ml_dtypes

    def bf(a):
        return np.asarray(a, np.float32).astype(ml_dtypes.bfloat16)

    d = _host_weights(inputs)
    base = {
        "w2qh": bf(np.ascontiguousarray(d["w2t"][:, :256])),
        "w2ql": bf(np.ascontiguousarray(d["w2t"][:, :256])
                   - np.asarray(bf(np.ascontiguousarray(d["w2t"][:, :256])),
                                np.float32)),
        "bi2": d["bi2"].astype(np.float32),
        "fw3": d["fw3"].astype(ml_dtypes.float8_e4m3),
        "fw5": d["fw5"].astype(ml_dtypes.float8_e4m3),
        "bc3": d["bc3"].astype(np.float32), "bc5": d["bc5"].astype(np.float32),
        "idt": bf(d["idt"]),
        "mp0": bf(d["mp0"]), "mp1": bf(d["mp1"]),
        "er": bf(d["er"]), "mdex": bf(d["mdex"]),
        "pw8": d["pw8"].astype(ml_dtypes.float8_e4m3),
        "pwid": bf(d["pwid"]),
        "pb": d["pb"].astype(np.float32),
        "svec": d["svec"].astype(np.float32),
        "w18": d["w18"].astype(ml_dtypes.float8_e4m3),
        "w2kv": bf(np.ascontiguousarray(d["w2t"][:, 256:])),
    }
    x = np.asarray(inputs["x"], np.float32).reshape(B, CIN, N)
    return base, x


def kernel(**inputs):
    from concourse.bass_utils import run_bass_kernel_spmd

    base, x = _feeds(inputs)
    in_maps = []
    for c in range(B):
        m = dict(base)
        m["xf"] = np.ascontiguousarray(x[c])
        in_maps.append(m)
    nc = _get_nc()
    res = run_bass_kernel_spmd(nc, in_maps, list(range(B))).results
    out = np.stack([np.asarray(r["out"]).reshape(CIN, HH, WW) for r in res])
    return out.astype(np.float32)

